# revision 10
# baseline (speedup 1.0000x reference)
"""Deformable conv net kernel for 8 TRN2 NeuronCores (data-parallel over batch).

Algorithm (per core, one batch sample):
  1. offsets = conv3x3(x, offset_w) + offset_b            (PE, bf16)
  2. per-pixel bilinear fields: corner indices + weights  (DVE, fp32)
  3. Y_k = W_k^T @ x for each of 9 taps (1x1 convs)       (PE, bf16)
     -- bilinear sampling commutes with the per-pixel 1x1 contraction,
        so we matmul first and gather afterwards.
  4. gather Y_k rows at the 4 corner indices              (SWDGE dma_gather)
  5. out[pix, o] = sum_{k,m} w_{k,m}[pix] * gath[pix, o]  (DVE tensor_scalar
     + PE identity-matmul accumulation into PSUM)
  6. out += bias; host reassembles [8, 128, 64, 64].
"""
import os, sys

for _p in ("/opt/trn_rl_repo", "/root/.axon_site/_ro/trn_rl_repo"):
    if os.path.isdir(_p) and _p not in sys.path:
        sys.path.insert(0, _p)

import numpy as np
import ml_dtypes

import concourse.bass as bass
import concourse.mybir as mybir
from concourse import bacc, library_config
from concourse.tile import TileContext

BF16 = mybir.dt.bfloat16
F32 = mybir.dt.float32
I16 = mybir.dt.int16

B, C, H, W = 8, 128, 64, 64
O = 128
K = 3
K2 = 9
HW = H * W                 # 4096
NCH = HW // 128            # 32 pixel chunks of 128
NG = 2                     # pixel groups for the gather phase
CLG = NCH // NG            # 16 chunks per group
MAGIC = float(3 * 2 ** 22)  # 1.5*2^23: keeps s+M in the ulp=1 binade

_MAX_WAITS = 1             # this walrus build rejects >1 sem wait per inst


def _split_excess_waits(nc):
    for f in nc.m.functions:
        for bb in f.blocks:
            new_insts = []
            for inst in bb.instructions:
                si = inst.sync_info
                if si is not None and si.on_wait and len(si.on_wait) > _MAX_WAITS:
                    waits = list(si.on_wait)
                    keep = waits[-_MAX_WAITS:]
                    spill = waits[:-_MAX_WAITS]
                    for j in range(0, len(spill), _MAX_WAITS):
                        chunk = spill[j:j + _MAX_WAITS]
                        nop = mybir.InstNoOp(
                            name=f"{inst.name}-wsp{j}",
                            engine=inst.engine,
                            ins=[], outs=[],
                            sync_info=mybir.SyncInfo(on_wait=chunk, on_update=[]),
                        )
                        nc.register_instruction(nop, overwrite=True)
                        new_insts.append(nop)
                    inst.sync_info = mybir.SyncInfo(
                        on_wait=keep, on_update=list(si.on_update or []))
                new_insts.append(inst)
            bb.instructions[:] = new_insts


def build_nc():
    nc = bacc.Bacc()
    x_in = nc.dram_tensor("x_img", [C, HW], BF16, kind="ExternalInput")
    offw_in = nc.dram_tensor("offw", [C, K2 * 18], BF16, kind="ExternalInput")
    offb_in = nc.dram_tensor("offb", [18, 1], F32, kind="ExternalInput")
    wmain_in = nc.dram_tensor("wmain", [C, K2 * O], BF16, kind="ExternalInput")
    bias_in = nc.dram_tensor("bias_t", [128, O], F32, kind="ExternalInput")
    ybase_in = nc.dram_tensor("ybase", [128, NCH * K2], F32, kind="ExternalInput")
    xbase_in = nc.dram_tensor("xbase", [128, NCH * K2], F32, kind="ExternalInput")
    idf_in = nc.dram_tensor("identf", [128, 128], F32, kind="ExternalInput")
    idb_in = nc.dram_tensor("identb", [128, 128], BF16, kind="ExternalInput")
    y_dram = nc.dram_tensor("y_scratch", [K2, HW, O], BF16)
    out_dram = nc.dram_tensor("out", [HW, O], F32, kind="ExternalOutput")

    FDIM = NCH * K2  # 288, (c, k) col = c*9 + k

    with TileContext(nc) as tc:
        with tc.tile_pool(name="cst", bufs=1) as cst, \
             tc.tile_pool(name="sb", bufs=2) as sb, \
             tc.tile_pool(name="fld", bufs=1) as fld, \
             tc.tile_pool(name="gth", bufs=3) as gth, \
             tc.tile_pool(name="tmp", bufs=4) as tmppool, \
             tc.tile_pool(name="ps", bufs=2, space="PSUM") as psp, \
             tc.tile_pool(name="pso", bufs=1, space="PSUM") as psop:

            nc.gpsimd.load_library(library_config.mlp)

            # ---- constant / input loads ----
            offw_sb = cst.tile([C, K2 * 18], BF16, name="offw_sb")
            nc.sync.dma_start(offw_sb[:, :], offw_in[:, :])
            wmain_sb = cst.tile([C, K2 * O], BF16, name="wmain_sb")
            nc.sync.dma_start(wmain_sb[:, :], wmain_in[:, :])
            offb_sb = cst.tile([18, 1], F32, name="offb_sb")
            nc.sync.dma_start(offb_sb[:, :], offb_in[:, :])
            bias_sb = cst.tile([128, O], F32, name="bias_sb")
            nc.sync.dma_start(bias_sb[:, :], bias_in[:, :])
            ybase_sb = cst.tile([128, FDIM], F32, name="ybase_sb")
            nc.sync.dma_start(ybase_sb[:, :], ybase_in[:, :])
            xbase_sb = cst.tile([128, FDIM], F32, name="xbase_sb")
            nc.sync.dma_start(xbase_sb[:, :], xbase_in[:, :])
            identf = cst.tile([128, 128], F32, name="identf")
            nc.sync.dma_start(identf[:, :], idf_in[:, :])
            identb = cst.tile([128, 128], BF16, name="identb")
            nc.sync.dma_start(identb[:, :], idb_in[:, :])

            # ---- contiguous input (for 1x1-conv lhsT) ----
            xin_sb = cst.tile([C, HW], BF16, name="xin_sb")
            nc.sync.dma_start(xin_sb[:, :], x_in[:, :])

            # ---- padded input image (zero border), bf16 ----
            HP, WP = H + 2, W + 2
            xpad = cst.tile([C, HP * WP], BF16, name="xpad")
            nc.vector.memset(xpad[:, :], 0.0)
            xpv = xpad[:, :].rearrange("c (r q) -> c r q", q=WP)
            nc.sync.dma_start(
                xpv[:, 1:HP - 1, 1:WP - 1],
                x_in[:, :].rearrange("c (y x) -> c y x", x=W))

            # ---- offset conv: offsets [18, HW] fp32 ----
            off_sb = fld.tile([18, HW], F32, name="off_sb")
            for r in range(H):  # one image row at a time
                off_ps = psp.tile([18, W], F32, name=f"offps{r}", tag="ph1ps")
                for k in range(K2):
                    kh, kw = k // 3, k % 3
                    rhs = xpv[:, r + kh, kw: kw + W]
                    nc.tensor.matmul(off_ps[:, :], offw_sb[:, k * 18:(k + 1) * 18],
                                     rhs, start=(k == 0), stop=(k == K2 - 1))
                nc.vector.tensor_scalar_add(off_sb[:, r * W:(r + 1) * W],
                                            off_ps[:, :], offb_sb[:, 0:1])

            # ---- transpose offsets to pixel-major: offT [128, 32*18] ----
            offT = fld.tile([128, NCH * 18], F32, name="offT")
            for c in range(NCH):
                tr_ps = psp.tile([128, 18], F32, name=f"trps{c}", tag="ph1ps")
                nc.tensor.transpose(tr_ps[:, :], off_sb[:, c * 128:(c + 1) * 128],
                                    identf[:18, :18])
                nc.scalar.copy(offT[:, c * 18:(c + 1) * 18], tr_ps[:, :])

            # ---- bilinear fields (fp32, [128, (c,k)=288]) ----
            offT4 = offT[:, :].rearrange("p (c k two) -> p two c k", two=2, k=K2)
            yb3 = ybase_sb[:, :].rearrange("p (c k) -> p c k", k=K2)
            xb3 = xbase_sb[:, :].rearrange("p (c k) -> p c k", k=K2)

            def f3(name):
                t = fld.tile([128, FDIM], F32, name=name, tag=name)
                return t, t[:, :].rearrange("p (c k) -> p c k", k=K2)

            VA = mybir.AluOpType
            axes = {}
            for ax in ("y", "x"):
                s, s3 = f3(f"s_{ax}")
                base3 = yb3 if ax == "y" else xb3
                nc.vector.tensor_tensor(s3, offT4[:, 0 if ax == "y" else 1], base3, VA.add)
                r, r3 = f3(f"r_{ax}")
                nc.vector.tensor_scalar_add(r[:, :], s[:, :], MAGIC)
                nc.vector.tensor_scalar_add(r[:, :], r[:, :], -MAGIC)
                g, g3 = f3(f"g_{ax}")
                nc.vector.tensor_tensor(g[:, :], r[:, :], s[:, :], VA.is_gt)
                i0, _ = f3(f"i0_{ax}")
                nc.vector.tensor_tensor(i0[:, :], r[:, :], g[:, :], VA.subtract)
                fr, _ = f3(f"fr_{ax}")
                nc.vector.tensor_tensor(fr[:, :], s[:, :], i0[:, :], VA.subtract)
                i1, _ = f3(f"i1_{ax}")
                nc.vector.tensor_scalar_add(i1[:, :], i0[:, :], 1.0)
                w_m = []
                for (ii, frac_is_w) in ((i0, False), (i1, True)):
                    v, _ = f3(f"v_{ax}_{frac_is_w}")
                    nc.vector.tensor_scalar(v[:, :], ii[:, :], 0.0, None, VA.is_ge)
                    t2, _ = f3(f"t2_{ax}_{frac_is_w}")
                    nc.vector.tensor_scalar(t2[:, :], ii[:, :], float(H - 1), None, VA.is_le)
                    nc.vector.tensor_tensor(v[:, :], v[:, :], t2[:, :], VA.mult)
                    wm, _ = f3(f"wm_{ax}_{frac_is_w}")
                    if frac_is_w:
                        nc.vector.tensor_tensor(wm[:, :], fr[:, :], v[:, :], VA.mult)
                    else:
                        nc.vector.tensor_scalar(wm[:, :], fr[:, :], -1.0, 1.0,
                                                VA.mult, VA.add)
                        nc.vector.tensor_tensor(wm[:, :], wm[:, :], v[:, :], VA.mult)
                    w_m.append(wm)
                cl = []
                for ii in (i0, i1):
                    cc, _ = f3(f"c_{ax}_{ii is i1}")
                    nc.vector.tensor_scalar(cc[:, :], ii[:, :], 0.0, float(H - 1),
                                            VA.max, VA.min)
                    cl.append(cc)
                axes[ax] = (w_m, cl)

            (wy, cy), (wx, cx) = axes["y"], axes["x"]
            # corner weights W_m, m = a*2 + b  (a: y corner, b: x corner)
            wcor = []
            for a in range(2):
                for b2 in range(2):
                    wc, _ = f3(f"wc{a}{b2}")
                    nc.vector.tensor_tensor(wc[:, :], wy[a][:, :], wx[b2][:, :], VA.mult)
                    wcor.append(wc)
            # gather row indices  idx = cy*64 + cx
            cys = []
            for a in range(2):
                cs, _ = f3(f"cys{a}")
                nc.vector.tensor_scalar_mul(cs[:, :], cy[a][:, :], float(W))
                cys.append(cs)
            # fidx col = ((k*2+g)*4+m)*16+cl  (k,g,m,cl ordering)
            fidx = fld.tile([128, 4 * FDIM], F32, name="fidx")
            fidx_r = fidx[:, :].rearrange("p (k g m cl) -> p m g cl k",
                                          k=K2, g=NG, m=4, cl=CLG)
            for a in range(2):
                for b2 in range(2):
                    m = a * 2 + b2
                    nc.vector.tensor_tensor(fidx_r[:, m],
                                            cys[a][:, :].rearrange(
                                                "p (g cl k) -> p g cl k",
                                                g=NG, cl=CLG, k=K2),
                                            cx[b2][:, :].rearrange(
                                                "p (g cl k) -> p g cl k",
                                                g=NG, cl=CLG, k=K2), VA.add)
            fidxi = fld.tile([128, 4 * FDIM], I16, name="fidxi")
            nc.vector.tensor_copy(fidxi[:, :], fidx[:, :])

            # ---- fold indices into SWDGE wrapped layout ----
            # idxw col = k*1024 + g*512 + m*128 + cl*8 + f ; value stream for
            # (k,g): i = m*2048 + cl*128 + 16f + p'  ->  (i%16, i//16)
            idxw = fld.tile([128, K2 * NG * 4 * CLG * 8], I16, name="idxw")
            src_r = fidxi[:, :].rearrange("p (k gmcl) -> p k gmcl",
                                          k=K2, gmcl=128)
            dst_r = idxw[:, :].rearrange("p (k gmcl f) -> p f k gmcl",
                                         k=K2, gmcl=128, f=8)
            for f in range(8):
                nc.sync.dma_start(dst_r[0:16, f],
                                  src_r[16 * f:16 * (f + 1)])
            for f in range(1, 8):
                nc.sync.dma_start(idxw[16 * f:16 * (f + 1), :], idxw[0:16, :])

            # ---- per-tap 1x1 convs:  yT[pix, (k,o)] = x_chunk^T @ wmain ----
            for c in range(NCH):
                lhs = xin_sb[:, c * 128:(c + 1) * 128]
                y_sb = sb.tile([128, K2 * O], BF16, name=f"ysb{c}", tag="ysb")
                for j in range(3):
                    y_ps = psp.tile([128, 384], F32, name=f"yps{c}_{j}", tag="yps")
                    nc.tensor.matmul(y_ps[:, :], lhs,
                                     wmain_sb[:, j * 384:(j + 1) * 384],
                                     start=True, stop=True)
                    if c % 2 == 0:
                        nc.scalar.copy(y_sb[:, j * 384:(j + 1) * 384], y_ps[:, :])
                    else:
                        nc.vector.tensor_copy(y_sb[:, j * 384:(j + 1) * 384], y_ps[:, :])
                nc.sync.dma_start(
                    y_dram[:, c * 128:(c + 1) * 128, :].rearrange("k p o -> p k o"),
                    y_sb[:, :].rearrange("p (k o) -> p k o", o=O))

            # ---- gather + weighted accumulate ----
            for g in range(NG):
                psob = []
                for q in range(CLG // 4):
                    p = psop.tile([128, 512], F32, name=f"pso{g}_{q}", tag=f"pso{q}")
                    psob.append(p)
                psout = [psob[cl // 4][:, (cl % 4) * O:(cl % 4 + 1) * O]
                         for cl in range(CLG)]
                for k in range(K2):
                    gt = gth.tile([128, 4 * CLG, O], BF16, name=f"g{g}_{k}", tag="gath")
                    for s in range(8):  # <=1024 descriptors per SWDGE inst
                        nc.gpsimd.dma_gather(
                            gt[:, s * 8:(s + 1) * 8, :], y_dram[k, :, :],
                            idxw[:, (k * NG + g) * 512 + s * 64:
                                 (k * NG + g) * 512 + (s + 1) * 64],
                            1024, 1024, O)
                    for m in range(4):
                        for q in range(CLG // 4):
                            tmp = tmppool.tile([128, 512], BF16,
                                               name=f"t{g}_{k}_{m}_{q}", tag="tmp")
                            for j in range(4):
                                cl = q * 4 + j
                                c = g * CLG + cl
                                nc.vector.tensor_scalar_mul(
                                    tmp[:, j * O:(j + 1) * O],
                                    gt[:, m * CLG + cl, :],
                                    wcor[m][:, c * K2 + k: c * K2 + k + 1])
                            nc.tensor.matmul(psob[q][:, :], identb[:, :], tmp[:, :],
                                             start=(k == 0 and m == 0),
                                             stop=(k == K2 - 1 and m == 3))
                for cl in range(CLG):
                    c = g * CLG + cl
                    ot = sb.tile([128, O], F32, name=f"o{g}_{cl}", tag="ot")
                    nc.vector.tensor_tensor(ot[:, :], psout[cl],
                                            bias_sb[:, :], mybir.AluOpType.add)
                    nc.sync.dma_start(out_dram[c * 128:(c + 1) * 128, :], ot[:, :])

    nc.compile()
    _split_excess_waits(nc)
    return nc


_NC_CACHE = None


def _get_nc():
    global _NC_CACHE
    if _NC_CACHE is None:
        _NC_CACHE = build_nc()
    return _NC_CACHE


def _host_inputs(x, offset_w, offset_b, weight, bias):
    bf = ml_dtypes.bfloat16
    # constant (shared) tensors
    offw = np.ascontiguousarray(
        offset_w.reshape(18, C, K2).transpose(1, 2, 0).reshape(C, K2 * 18)).astype(bf)
    wmain = np.ascontiguousarray(
        weight.reshape(O, C, K2).transpose(1, 2, 0).reshape(C, K2 * O)).astype(bf)
    offb = offset_b.reshape(18, 1).astype(np.float32)
    bias_t = np.tile(bias.reshape(1, O), (128, 1)).astype(np.float32)
    pi = np.arange(128)
    cc = np.arange(NCH)
    kk = np.arange(K2)
    pix = cc[None, :, None] * 128 + pi[:, None, None]          # [128, 32, 1]
    ybase = (pix // W - 1 + (kk // 3)[None, None, :]).reshape(128, FDIM_np).astype(np.float32)
    xbase = (pix % W - 1 + (kk % 3)[None, None, :]).reshape(128, FDIM_np).astype(np.float32)
    identf = np.eye(128, dtype=np.float32)
    identb = np.eye(128, dtype=bf)
    shared = dict(offw=offw, wmain=wmain, offb=offb, bias_t=bias_t,
                  ybase=ybase, xbase=xbase, identf=identf, identb=identb)
    maps = []
    for b in range(B):
        m = dict(shared)
        m["x_img"] = np.ascontiguousarray(x[b].reshape(C, HW)).astype(bf)
        maps.append(m)
    return maps


FDIM_np = NCH * K2


def kernel(x, offset_w, offset_b, weight, bias):
    from concourse.bass_utils import run_bass_kernel_spmd
    nc = _get_nc()
    in_maps = _host_inputs(np.asarray(x, np.float32), np.asarray(offset_w, np.float32),
                           np.asarray(offset_b, np.float32),
                           np.asarray(weight, np.float32), np.asarray(bias, np.float32))
    res = run_bass_kernel_spmd(nc, in_maps, core_ids=list(range(B)))
    out = np.stack([np.asarray(res.results[b]["out"], np.float32).T.reshape(O, H, W)
                    for b in range(B)])
    return out


# revision 13
# speedup vs baseline: 2928.3593x; 2928.3593x over previous
"""Deformable conv net kernel for 8 TRN2 NeuronCores (data-parallel over batch).

Algorithm (per core, one batch sample):
  1. offsets = conv3x3(x, offset_w) + offset_b            (PE, bf16)
  2. per-pixel bilinear fields: corner indices + weights  (DVE, fp32)
  3. Y_k = W_k^T @ x for each of 9 taps (1x1 convs)       (PE, bf16)
     -- bilinear sampling commutes with the per-pixel 1x1 contraction,
        so we matmul first and gather afterwards.
  4. gather Y_k rows at the 4 corner indices              (SWDGE dma_gather)
  5. out[pix, o] = sum_{k,m} w_{k,m}[pix] * gath[pix, o]  (DVE tensor_scalar
     + PE identity-matmul accumulation into PSUM)
  6. out += bias; host reassembles [8, 128, 64, 64].
"""
import os, sys

for _p in ("/opt/trn_rl_repo", "/root/.axon_site/_ro/trn_rl_repo"):
    if os.path.isdir(_p) and _p not in sys.path:
        sys.path.insert(0, _p)

import numpy as np
import ml_dtypes

import concourse.bass as bass
import concourse.mybir as mybir
from concourse import bacc, library_config
from concourse.tile import TileContext

BF16 = mybir.dt.bfloat16
F32 = mybir.dt.float32
I16 = mybir.dt.int16

B, C, H, W = 8, 128, 64, 64
O = 128
K = 3
K2 = 9
HW = H * W                 # 4096
NCH = HW // 128            # 32 pixel chunks of 128
NG = 2                     # pixel groups for the gather phase
CLG = NCH // NG            # 16 chunks per group
MAGIC = float(3 * 2 ** 22)  # 1.5*2^23: keeps s+M in the ulp=1 binade

_MAX_WAITS = 1             # this walrus build rejects >1 sem wait per inst


def _split_excess_waits(nc):
    for f in nc.m.functions:
        for bb in f.blocks:
            new_insts = []
            for inst in bb.instructions:
                si = inst.sync_info
                if si is not None and si.on_wait and len(si.on_wait) > _MAX_WAITS:
                    waits = list(si.on_wait)
                    keep = waits[-_MAX_WAITS:]
                    spill = waits[:-_MAX_WAITS]
                    for j in range(0, len(spill), _MAX_WAITS):
                        chunk = spill[j:j + _MAX_WAITS]
                        nop = mybir.InstNoOp(
                            name=f"{inst.name}-wsp{j}",
                            engine=inst.engine,
                            ins=[], outs=[],
                            sync_info=mybir.SyncInfo(on_wait=chunk, on_update=[]),
                        )
                        nc.register_instruction(nop, overwrite=True)
                        new_insts.append(nop)
                    inst.sync_info = mybir.SyncInfo(
                        on_wait=keep, on_update=list(si.on_update or []))
                new_insts.append(inst)
            bb.instructions[:] = new_insts


def build_nc():
    nc = bacc.Bacc()
    x_in = nc.dram_tensor("x_img", [C, HW], BF16, kind="ExternalInput")
    offw_in = nc.dram_tensor("offw", [C, K2 * 18], BF16, kind="ExternalInput")
    offb_in = nc.dram_tensor("offb", [18, 1], F32, kind="ExternalInput")
    wmain_in = nc.dram_tensor("wmain", [C, K2 * O], BF16, kind="ExternalInput")
    bias_in = nc.dram_tensor("bias_t", [128, O], F32, kind="ExternalInput")
    ybase_in = nc.dram_tensor("ybase", [128, NCH * K2], F32, kind="ExternalInput")
    xbase_in = nc.dram_tensor("xbase", [128, NCH * K2], F32, kind="ExternalInput")
    idf_in = nc.dram_tensor("identf", [128, 128], F32, kind="ExternalInput")
    idb_in = nc.dram_tensor("identb", [128, 128], BF16, kind="ExternalInput")
    y_dram = nc.dram_tensor("y_scratch", [K2, HW, O], BF16)
    out_dram = nc.dram_tensor("out", [HW, O], F32, kind="ExternalOutput")

    FDIM = NCH * K2  # 288, (c, k) col = c*9 + k

    with TileContext(nc) as tc:
        with tc.tile_pool(name="cst", bufs=1) as cst, \
             tc.tile_pool(name="sb", bufs=2) as sb, \
             tc.tile_pool(name="fld", bufs=1) as fld, \
             tc.tile_pool(name="gth", bufs=3) as gth, \
             tc.tile_pool(name="tmp", bufs=4) as tmppool, \
             tc.tile_pool(name="ps", bufs=2, space="PSUM") as psp, \
             tc.tile_pool(name="pso", bufs=1, space="PSUM") as psop:

            nc.gpsimd.load_library(library_config.mlp)

            # ---- constant / input loads ----
            offw_sb = cst.tile([C, K2 * 18], BF16, name="offw_sb")
            nc.sync.dma_start(offw_sb[:, :], offw_in[:, :])
            wmain_sb = cst.tile([C, K2 * O], BF16, name="wmain_sb")
            nc.sync.dma_start(wmain_sb[:, :], wmain_in[:, :])
            offb_sb = cst.tile([18, 1], F32, name="offb_sb")
            nc.sync.dma_start(offb_sb[:, :], offb_in[:, :])
            bias_sb = cst.tile([128, O], F32, name="bias_sb")
            nc.sync.dma_start(bias_sb[:, :], bias_in[:, :])
            ybase_sb = cst.tile([128, FDIM], F32, name="ybase_sb")
            nc.sync.dma_start(ybase_sb[:, :], ybase_in[:, :])
            xbase_sb = cst.tile([128, FDIM], F32, name="xbase_sb")
            nc.sync.dma_start(xbase_sb[:, :], xbase_in[:, :])
            identf = cst.tile([128, 128], F32, name="identf")
            nc.sync.dma_start(identf[:, :], idf_in[:, :])
            identb = cst.tile([128, 128], BF16, name="identb")
            nc.sync.dma_start(identb[:, :], idb_in[:, :])

            # ---- contiguous input (for 1x1-conv lhsT) ----
            xin_sb = cst.tile([C, HW], BF16, name="xin_sb")
            nc.sync.dma_start(xin_sb[:, :], x_in[:, :])

            # ---- padded input image (zero border), bf16 ----
            HP, WP = H + 2, W + 2
            xpad = cst.tile([C, HP * WP], BF16, name="xpad")
            nc.vector.memset(xpad[:, :], 0.0)
            xpv = xpad[:, :].rearrange("c (r q) -> c r q", q=WP)
            nc.sync.dma_start(
                xpv[:, 1:HP - 1, 1:WP - 1],
                x_in[:, :].rearrange("c (y x) -> c y x", x=W))

            # ---- offset conv: offsets [18, HW] fp32 ----
            off_sb = fld.tile([18, HW], F32, name="off_sb")
            for r in range(H):  # one image row at a time
                off_ps = psp.tile([18, W], F32, name=f"offps{r}", tag="ph1ps")
                for k in range(K2):
                    kh, kw = k // 3, k % 3
                    rhs = xpv[:, r + kh, kw: kw + W]
                    nc.tensor.matmul(off_ps[:, :], offw_sb[:, k * 18:(k + 1) * 18],
                                     rhs, start=(k == 0), stop=(k == K2 - 1))
                nc.vector.tensor_scalar_add(off_sb[:, r * W:(r + 1) * W],
                                            off_ps[:, :], offb_sb[:, 0:1])

            # ---- transpose offsets to pixel-major: offT [128, 32*18] ----
            offT = fld.tile([128, NCH * 18], F32, name="offT")
            for c in range(NCH):
                tr_ps = psp.tile([128, 18], F32, name=f"trps{c}", tag="ph1ps")
                nc.tensor.transpose(tr_ps[:, :], off_sb[:, c * 128:(c + 1) * 128],
                                    identf[:18, :18])
                nc.scalar.copy(offT[:, c * 18:(c + 1) * 18], tr_ps[:, :])

            # ---- bilinear fields (fp32, [128, (c,k)=288]) ----
            offT4 = offT[:, :].rearrange("p (c k two) -> p two c k", two=2, k=K2)
            yb3 = ybase_sb[:, :].rearrange("p (c k) -> p c k", k=K2)
            xb3 = xbase_sb[:, :].rearrange("p (c k) -> p c k", k=K2)

            def f3(name):
                t = fld.tile([128, FDIM], F32, name=name, tag=name)
                return t, t[:, :].rearrange("p (c k) -> p c k", k=K2)

            VA = mybir.AluOpType
            axes = {}
            axes_i0 = {}
            for ax in ("y", "x"):
                s, s3 = f3(f"s_{ax}")
                base3 = yb3 if ax == "y" else xb3
                nc.vector.tensor_tensor(s3, offT4[:, 0 if ax == "y" else 1], base3, VA.add)
                r, r3 = f3(f"r_{ax}")
                nc.vector.tensor_scalar_add(r[:, :], s[:, :], MAGIC)
                nc.vector.tensor_scalar_add(r[:, :], r[:, :], -MAGIC)
                g, g3 = f3(f"g_{ax}")
                nc.vector.tensor_tensor(g[:, :], r[:, :], s[:, :], VA.is_gt)
                i0, _ = f3(f"i0_{ax}")
                nc.vector.tensor_tensor(i0[:, :], r[:, :], g[:, :], VA.subtract)
                fr, _ = f3(f"fr_{ax}")
                nc.vector.tensor_tensor(fr[:, :], s[:, :], i0[:, :], VA.subtract)
                i1, _ = f3(f"i1_{ax}")
                nc.vector.tensor_scalar_add(i1[:, :], i0[:, :], 1.0)
                w_m = []
                for (ii, frac_is_w) in ((i0, False), (i1, True)):
                    v, _ = f3(f"v_{ax}_{frac_is_w}")
                    nc.vector.tensor_scalar(v[:, :], ii[:, :], 0.0, None, VA.is_ge)
                    t2, _ = f3(f"t2_{ax}_{frac_is_w}")
                    nc.vector.tensor_scalar(t2[:, :], ii[:, :], float(H - 1), None, VA.is_le)
                    nc.vector.tensor_tensor(v[:, :], v[:, :], t2[:, :], VA.mult)
                    wm, _ = f3(f"wm_{ax}_{frac_is_w}")
                    if frac_is_w:
                        nc.vector.tensor_tensor(wm[:, :], fr[:, :], v[:, :], VA.mult)
                    else:
                        nc.vector.tensor_scalar(wm[:, :], fr[:, :], -1.0, 1.0,
                                                VA.mult, VA.add)
                        nc.vector.tensor_tensor(wm[:, :], wm[:, :], v[:, :], VA.mult)
                    w_m.append(wm)
                cl = []
                for ii in (i0, i1):
                    cc, _ = f3(f"c_{ax}_{ii is i1}")
                    nc.vector.tensor_scalar(cc[:, :], ii[:, :], 0.0, float(H - 1),
                                            VA.max, VA.min)
                    cl.append(cc)
                axes[ax] = (w_m, cl)
                axes_i0[ax] = i0

            (wy, cy), (wx, _cxunused) = axes["y"], axes["x"]
            # pair-fetch base bx = clip(ix0, 0, 62); weights for pair slots
            ix0f = axes_i0["x"]
            bx, _ = f3("bx")
            nc.vector.tensor_scalar(bx[:, :], ix0f[:, :], 0.0, float(W - 2),
                                    VA.max, VA.min)
            dif, _ = f3("dif")
            nc.vector.tensor_tensor(dif[:, :], bx[:, :], ix0f[:, :], VA.subtract)
            eqA, _ = f3("eqA")
            nc.vector.tensor_scalar(eqA[:, :], dif[:, :], 0.0, None, VA.is_equal)
            eqB, _ = f3("eqB")
            nc.vector.tensor_scalar(eqB[:, :], dif[:, :], 1.0, None, VA.is_equal)
            eqC, _ = f3("eqC")
            nc.vector.tensor_scalar(eqC[:, :], dif[:, :], -1.0, None, VA.is_equal)
            WL, _ = f3("WL")
            WR, _ = f3("WR")
            t1, _ = f3("t1")
            nc.vector.tensor_tensor(WL[:, :], wx[0][:, :], eqA[:, :], VA.mult)
            nc.vector.tensor_tensor(t1[:, :], wx[1][:, :], eqB[:, :], VA.mult)
            nc.vector.tensor_tensor(WL[:, :], WL[:, :], t1[:, :], VA.add)
            nc.vector.tensor_tensor(WR[:, :], wx[1][:, :], eqA[:, :], VA.mult)
            nc.vector.tensor_tensor(t1[:, :], wx[0][:, :], eqC[:, :], VA.mult)
            nc.vector.tensor_tensor(WR[:, :], WR[:, :], t1[:, :], VA.add)
            # weights per (a, side): wcor2[a*2+side]
            wcor2 = []
            for a in range(2):
                for sd, Wside in ((0, WL), (1, WR)):
                    wc, _ = f3(f"wc{a}{sd}")
                    nc.vector.tensor_tensor(wc[:, :], wy[a][:, :], Wside[:, :], VA.mult)
                    wcor2.append(wc)
            # pair row indices idx = cy*64 + bx; fidx col = k*64+g*32+a*16+cl
            cys = []
            for a in range(2):
                cs, _ = f3(f"cys{a}")
                nc.vector.tensor_scalar_mul(cs[:, :], cy[a][:, :], float(W))
                cys.append(cs)
            fidx = fld.tile([128, 2 * FDIM], F32, name="fidx")
            fidx_r = fidx[:, :].rearrange("p (k g a cl) -> p a g cl k",
                                          k=K2, g=NG, a=2, cl=CLG)
            for a in range(2):
                nc.vector.tensor_tensor(fidx_r[:, a],
                                        cys[a][:, :].rearrange(
                                            "p (g cl k) -> p g cl k",
                                            g=NG, cl=CLG, k=K2),
                                        bx[:, :].rearrange(
                                            "p (g cl k) -> p g cl k",
                                            g=NG, cl=CLG, k=K2), VA.add)
            fidxi = fld.tile([128, 2 * FDIM], I16, name="fidxi")
            nc.vector.tensor_copy(fidxi[:, :], fidx[:, :])

            # ---- fold indices into SWDGE wrapped layout ----
            # idxw col = k*1024 + g*512 + m*128 + cl*8 + f ; value stream for
            # (k,g): i = m*2048 + cl*128 + 16f + p'  ->  (i%16, i//16)
            idxw = fld.tile([128, K2 * NG * 2 * CLG * 8], I16, name="idxw")
            src_r = fidxi[:, :].rearrange("p (k gacl) -> p k gacl",
                                          k=K2, gacl=64)
            dst_r = idxw[:, :].rearrange("p (k gacl f) -> p f k gacl",
                                         k=K2, gacl=64, f=8)
            for f in range(8):
                nc.sync.dma_start(dst_r[0:16, f],
                                  src_r[16 * f:16 * (f + 1)])
            for f in range(1, 8):
                nc.sync.dma_start(idxw[16 * f:16 * (f + 1), :], idxw[0:16, :])

            # ---- per-tap 1x1 convs:  yT[pix, (k,o)] = x_chunk^T @ wmain ----
            for c in range(NCH):
                lhs = xin_sb[:, c * 128:(c + 1) * 128]
                y_sb = sb.tile([128, K2 * O], BF16, name=f"ysb{c}", tag="ysb")
                for j in range(3):
                    y_ps = psp.tile([128, 384], F32, name=f"yps{c}_{j}", tag="yps")
                    nc.tensor.matmul(y_ps[:, :], lhs,
                                     wmain_sb[:, j * 384:(j + 1) * 384],
                                     start=True, stop=True)
                    if c % 2 == 0:
                        nc.scalar.copy(y_sb[:, j * 384:(j + 1) * 384], y_ps[:, :])
                    else:
                        nc.vector.tensor_copy(y_sb[:, j * 384:(j + 1) * 384], y_ps[:, :])
                nc.sync.dma_start(
                    y_dram[:, c * 128:(c + 1) * 128, :].rearrange("k p o -> p k o"),
                    y_sb[:, :].rearrange("p (k o) -> p k o", o=O))

            # ---- gather + weighted accumulate ----
            for g in range(NG):
                psob = []
                for q in range(CLG // 4):
                    p = psop.tile([128, 512], F32, name=f"pso{g}_{q}", tag=f"pso{q}")
                    psob.append(p)
                psout = [psob[cl // 4][:, (cl % 4) * O:(cl % 4 + 1) * O]
                         for cl in range(CLG)]
                for k in range(K2):
                    ysrc = y_dram[k, :, :]
                    ypairs = bass.AP(tensor=ysrc.tensor, offset=ysrc.offset,
                                     ap=[[O, HW - 1], [1, 2 * O]])
                    for a in range(2):
                        gt = gth.tile([128, CLG, 2 * O], BF16,
                                      name=f"g{g}_{k}_{a}", tag="gath")
                        base = k * 512 + g * 256 + a * 128
                        for s in range(2):  # <=1024 descriptors per SWDGE inst
                            nc.gpsimd.dma_gather(
                                gt[:, s * 8:(s + 1) * 8, :], ypairs,
                                idxw[:, base + s * 64: base + (s + 1) * 64],
                                1024, 1024, 2 * O, elem_step=O)
                        for sd in range(2):
                            for q in range(CLG // 4):
                                tmp = tmppool.tile([128, 512], BF16,
                                                   name=f"t{g}_{k}_{a}_{sd}_{q}",
                                                   tag="tmp")
                                for j in range(4):
                                    cl = q * 4 + j
                                    c = g * CLG + cl
                                    nc.vector.tensor_scalar_mul(
                                        tmp[:, j * O:(j + 1) * O],
                                        gt[:, cl, sd * O:(sd + 1) * O],
                                        wcor2[a * 2 + sd][:, c * K2 + k:
                                                          c * K2 + k + 1])
                                nc.tensor.matmul(psob[q][:, :], identb[:, :],
                                                 tmp[:, :],
                                                 start=(k == 0 and a == 0 and sd == 0),
                                                 stop=(k == K2 - 1 and a == 1 and sd == 1))
                for cl in range(CLG):
                    c = g * CLG + cl
                    ot = sb.tile([128, O], F32, name=f"o{g}_{cl}", tag="ot")
                    nc.vector.tensor_tensor(ot[:, :], psout[cl],
                                            bias_sb[:, :], mybir.AluOpType.add)
                    nc.sync.dma_start(out_dram[c * 128:(c + 1) * 128, :], ot[:, :])

    nc.compile()
    _split_excess_waits(nc)
    return nc


_NC_CACHE = None


def _get_nc():
    global _NC_CACHE
    if _NC_CACHE is None:
        _NC_CACHE = build_nc()
    return _NC_CACHE


def _host_inputs(x, offset_w, offset_b, weight, bias):
    bf = ml_dtypes.bfloat16
    # constant (shared) tensors
    offw = np.ascontiguousarray(
        offset_w.reshape(18, C, K2).transpose(1, 2, 0).reshape(C, K2 * 18)).astype(bf)
    wmain = np.ascontiguousarray(
        weight.reshape(O, C, K2).transpose(1, 2, 0).reshape(C, K2 * O)).astype(bf)
    offb = offset_b.reshape(18, 1).astype(np.float32)
    bias_t = np.tile(bias.reshape(1, O), (128, 1)).astype(np.float32)
    pi = np.arange(128)
    cc = np.arange(NCH)
    kk = np.arange(K2)
    pix = cc[None, :, None] * 128 + pi[:, None, None]          # [128, 32, 1]
    ybase = (pix // W - 1 + (kk // 3)[None, None, :]).reshape(128, FDIM_np).astype(np.float32)
    xbase = (pix % W - 1 + (kk % 3)[None, None, :]).reshape(128, FDIM_np).astype(np.float32)
    identf = np.eye(128, dtype=np.float32)
    identb = np.eye(128, dtype=bf)
    shared = dict(offw=offw, wmain=wmain, offb=offb, bias_t=bias_t,
                  ybase=ybase, xbase=xbase, identf=identf, identb=identb)
    maps = []
    for b in range(B):
        m = dict(shared)
        m["x_img"] = np.ascontiguousarray(x[b].reshape(C, HW)).astype(bf)
        maps.append(m)
    return maps


FDIM_np = NCH * K2


def kernel(x, offset_w, offset_b, weight, bias):
    from concourse.bass_utils import run_bass_kernel_spmd
    nc = _get_nc()
    in_maps = _host_inputs(np.asarray(x, np.float32), np.asarray(offset_w, np.float32),
                           np.asarray(offset_b, np.float32),
                           np.asarray(weight, np.float32), np.asarray(bias, np.float32))
    res = run_bass_kernel_spmd(nc, in_maps, core_ids=list(range(B)))
    out = np.stack([np.asarray(res.results[b]["out"], np.float32).T.reshape(O, H, W)
                    for b in range(B)])
    return out


# revision 18
# speedup vs baseline: 3139.2761x; 1.0720x over previous
"""Deformable conv net kernel for 8 TRN2 NeuronCores (data-parallel over batch).

Algorithm (per core, one batch sample):
  1. offsets = conv3x3(x, offset_w) + offset_b            (PE, bf16)
  2. per-pixel bilinear fields: corner indices + weights  (DVE, fp32)
  3. Y_k = W_k^T @ x for each of 9 taps (1x1 convs)       (PE, bf16)
     -- bilinear sampling commutes with the per-pixel 1x1 contraction,
        so we matmul first and gather afterwards.
  4. gather Y_k rows at the 4 corner indices              (SWDGE dma_gather)
  5. out[pix, o] = sum_{k,m} w_{k,m}[pix] * gath[pix, o]  (DVE tensor_scalar
     + PE identity-matmul accumulation into PSUM)
  6. out += bias; host reassembles [8, 128, 64, 64].
"""
import os, sys

for _p in ("/opt/trn_rl_repo", "/root/.axon_site/_ro/trn_rl_repo"):
    if os.path.isdir(_p) and _p not in sys.path:
        sys.path.insert(0, _p)

import numpy as np
import ml_dtypes

import concourse.bass as bass
import concourse.mybir as mybir
from concourse import bacc, library_config
from concourse.tile import TileContext

BF16 = mybir.dt.bfloat16
F32 = mybir.dt.float32
I16 = mybir.dt.int16

B, C, H, W = 8, 128, 64, 64
O = 128
K = 3
K2 = 9
HW = H * W                 # 4096
NCH = HW // 128            # 32 pixel chunks of 128
NG = 2                     # pixel groups for the gather phase
CLG = NCH // NG            # 16 chunks per group
MAGIC = float(3 * 2 ** 22)  # 1.5*2^23: keeps s+M in the ulp=1 binade

_MAX_WAITS = 1             # this walrus build rejects >1 sem wait per inst


def _split_excess_waits(nc):
    for f in nc.m.functions:
        for bb in f.blocks:
            new_insts = []
            for inst in bb.instructions:
                si = inst.sync_info
                if si is not None and si.on_wait and len(si.on_wait) > _MAX_WAITS:
                    waits = list(si.on_wait)
                    keep = waits[-_MAX_WAITS:]
                    spill = waits[:-_MAX_WAITS]
                    for j in range(0, len(spill), _MAX_WAITS):
                        chunk = spill[j:j + _MAX_WAITS]
                        nop = mybir.InstNoOp(
                            name=f"{inst.name}-wsp{j}",
                            engine=inst.engine,
                            ins=[], outs=[],
                            sync_info=mybir.SyncInfo(on_wait=chunk, on_update=[]),
                        )
                        nc.register_instruction(nop, overwrite=True)
                        new_insts.append(nop)
                    inst.sync_info = mybir.SyncInfo(
                        on_wait=keep, on_update=list(si.on_update or []))
                new_insts.append(inst)
            bb.instructions[:] = new_insts


def build_nc(act_mod=0, ycopy_act=True):
    nc = bacc.Bacc()
    x_in = nc.dram_tensor("x_img", [C, HW], BF16, kind="ExternalInput")
    offw_in = nc.dram_tensor("offw", [C, K2 * 18], BF16, kind="ExternalInput")
    offb_in = nc.dram_tensor("offb", [18, 1], F32, kind="ExternalInput")
    wmain_in = nc.dram_tensor("wmain", [C, K2 * O], BF16, kind="ExternalInput")
    bias_in = nc.dram_tensor("bias_t", [128, O], F32, kind="ExternalInput")
    ybase_in = nc.dram_tensor("ybase", [128, NCH * K2], F32, kind="ExternalInput")
    xbase_in = nc.dram_tensor("xbase", [128, NCH * K2], F32, kind="ExternalInput")
    idf_in = nc.dram_tensor("identf", [128, 128], F32, kind="ExternalInput")
    idb_in = nc.dram_tensor("identb", [128, 128], BF16, kind="ExternalInput")
    y_dram = nc.dram_tensor("y_scratch", [K2, HW, O], BF16)
    out_dram = nc.dram_tensor("out", [HW, O], F32, kind="ExternalOutput")

    FDIM = NCH * K2  # 288, (c, k) col = c*9 + k

    with TileContext(nc) as tc:
        with tc.tile_pool(name="cst", bufs=1) as cst, \
             tc.tile_pool(name="sb", bufs=2) as sb, \
             tc.tile_pool(name="fld", bufs=1) as fld, \
             tc.tile_pool(name="gth", bufs=3) as gth, \
             tc.tile_pool(name="tmp", bufs=4) as tmppool, \
             tc.tile_pool(name="ps", bufs=2, space="PSUM") as psp, \
             tc.tile_pool(name="pso", bufs=1, space="PSUM") as psop:

            nc.gpsimd.load_library(library_config.mlp)

            # ---- constant / input loads ----
            offw_sb = cst.tile([C, K2 * 18], BF16, name="offw_sb")
            nc.sync.dma_start(offw_sb[:, :], offw_in[:, :])
            wmain_sb = cst.tile([C, K2 * O], BF16, name="wmain_sb")
            nc.sync.dma_start(wmain_sb[:, :], wmain_in[:, :])
            offb_sb = cst.tile([18, 1], F32, name="offb_sb")
            nc.sync.dma_start(offb_sb[:, :], offb_in[:, :])
            bias_sb = cst.tile([128, O], F32, name="bias_sb")
            nc.sync.dma_start(bias_sb[:, :], bias_in[:, :])
            ybase_sb = cst.tile([128, FDIM], F32, name="ybase_sb")
            nc.sync.dma_start(ybase_sb[:, :], ybase_in[:, :])
            xbase_sb = cst.tile([128, FDIM], F32, name="xbase_sb")
            nc.sync.dma_start(xbase_sb[:, :], xbase_in[:, :])
            identf = cst.tile([128, 128], F32, name="identf")
            nc.sync.dma_start(identf[:, :], idf_in[:, :])
            identb = cst.tile([128, 128], BF16, name="identb")
            nc.sync.dma_start(identb[:, :], idb_in[:, :])

            # ---- contiguous input (for 1x1-conv lhsT) ----
            xin_sb = cst.tile([C, HW], BF16, name="xin_sb")
            nc.sync.dma_start(xin_sb[:, :], x_in[:, :])

            # ---- padded input image (zero border), bf16 ----
            HP, WP = H + 2, W + 2
            xpad = cst.tile([C, HP * WP], BF16, name="xpad")
            nc.vector.memset(xpad[:, :], 0.0)
            xpv = xpad[:, :].rearrange("c (r q) -> c r q", q=WP)
            nc.sync.dma_start(
                xpv[:, 1:HP - 1, 1:WP - 1],
                x_in[:, :].rearrange("c (y x) -> c y x", x=W))

            # ---- offset conv: offsets [18, HW] fp32 ----
            off_sb = fld.tile([18, HW], F32, name="off_sb")
            for r in range(H):  # one image row at a time
                off_ps = psp.tile([18, W], F32, name=f"offps{r}", tag="ph1ps")
                for k in range(K2):
                    kh, kw = k // 3, k % 3
                    rhs = xpv[:, r + kh, kw: kw + W]
                    nc.tensor.matmul(off_ps[:, :], offw_sb[:, k * 18:(k + 1) * 18],
                                     rhs, start=(k == 0), stop=(k == K2 - 1))
                nc.vector.tensor_scalar_add(off_sb[:, r * W:(r + 1) * W],
                                            off_ps[:, :], offb_sb[:, 0:1])

            # ---- transpose offsets to pixel-major: offT [128, 32*18] ----
            offT = fld.tile([128, NCH * 18], F32, name="offT")
            for c in range(NCH):
                tr_ps = psp.tile([128, 18], F32, name=f"trps{c}", tag="ph1ps")
                nc.tensor.transpose(tr_ps[:, :], off_sb[:, c * 128:(c + 1) * 128],
                                    identf[:18, :18])
                nc.scalar.copy(offT[:, c * 18:(c + 1) * 18], tr_ps[:, :])

            # ---- bilinear fields (fp32, [128, (c,k)=288]) ----
            offT4 = offT[:, :].rearrange("p (c k two) -> p two c k", two=2, k=K2)
            yb3 = ybase_sb[:, :].rearrange("p (c k) -> p c k", k=K2)
            xb3 = xbase_sb[:, :].rearrange("p (c k) -> p c k", k=K2)

            def f3(name):
                t = fld.tile([128, FDIM], F32, name=name, tag=name)
                return t, t[:, :].rearrange("p (c k) -> p c k", k=K2)

            VA = mybir.AluOpType
            axes = {}
            axes_i0 = {}
            for ax in ("y", "x"):
                s, s3 = f3(f"s_{ax}")
                base3 = yb3 if ax == "y" else xb3
                nc.vector.tensor_tensor(s3, offT4[:, 0 if ax == "y" else 1], base3, VA.add)
                r, r3 = f3(f"r_{ax}")
                nc.vector.tensor_scalar_add(r[:, :], s[:, :], MAGIC)
                nc.vector.tensor_scalar_add(r[:, :], r[:, :], -MAGIC)
                g, g3 = f3(f"g_{ax}")
                nc.vector.tensor_tensor(g[:, :], r[:, :], s[:, :], VA.is_gt)
                i0, _ = f3(f"i0_{ax}")
                nc.vector.tensor_tensor(i0[:, :], r[:, :], g[:, :], VA.subtract)
                fr, _ = f3(f"fr_{ax}")
                nc.vector.tensor_tensor(fr[:, :], s[:, :], i0[:, :], VA.subtract)
                i1, _ = f3(f"i1_{ax}")
                nc.vector.tensor_scalar_add(i1[:, :], i0[:, :], 1.0)
                w_m = []
                for (ii, frac_is_w) in ((i0, False), (i1, True)):
                    v, _ = f3(f"v_{ax}_{frac_is_w}")
                    nc.vector.tensor_scalar(v[:, :], ii[:, :], 0.0, None, VA.is_ge)
                    t2, _ = f3(f"t2_{ax}_{frac_is_w}")
                    nc.vector.tensor_scalar(t2[:, :], ii[:, :], float(H - 1), None, VA.is_le)
                    nc.vector.tensor_tensor(v[:, :], v[:, :], t2[:, :], VA.mult)
                    wm, _ = f3(f"wm_{ax}_{frac_is_w}")
                    if frac_is_w:
                        nc.vector.tensor_tensor(wm[:, :], fr[:, :], v[:, :], VA.mult)
                    else:
                        nc.vector.tensor_scalar(wm[:, :], fr[:, :], -1.0, 1.0,
                                                VA.mult, VA.add)
                        nc.vector.tensor_tensor(wm[:, :], wm[:, :], v[:, :], VA.mult)
                    w_m.append(wm)
                cl = []
                for ii in (i0, i1):
                    cc, _ = f3(f"c_{ax}_{ii is i1}")
                    nc.vector.tensor_scalar(cc[:, :], ii[:, :], 0.0, float(H - 1),
                                            VA.max, VA.min)
                    cl.append(cc)
                axes[ax] = (w_m, cl)
                axes_i0[ax] = i0

            (wy, cy), (wx, _cxunused) = axes["y"], axes["x"]
            # pair-fetch base bx = clip(ix0, 0, 62); weights for pair slots
            ix0f = axes_i0["x"]
            bx, _ = f3("bx")
            nc.vector.tensor_scalar(bx[:, :], ix0f[:, :], 0.0, float(W - 2),
                                    VA.max, VA.min)
            dif, _ = f3("dif")
            nc.vector.tensor_tensor(dif[:, :], bx[:, :], ix0f[:, :], VA.subtract)
            eqA, _ = f3("eqA")
            nc.vector.tensor_scalar(eqA[:, :], dif[:, :], 0.0, None, VA.is_equal)
            eqB, _ = f3("eqB")
            nc.vector.tensor_scalar(eqB[:, :], dif[:, :], 1.0, None, VA.is_equal)
            eqC, _ = f3("eqC")
            nc.vector.tensor_scalar(eqC[:, :], dif[:, :], -1.0, None, VA.is_equal)
            WL, _ = f3("WL")
            WR, _ = f3("WR")
            t1, _ = f3("t1")
            nc.vector.tensor_tensor(WL[:, :], wx[0][:, :], eqA[:, :], VA.mult)
            nc.vector.tensor_tensor(t1[:, :], wx[1][:, :], eqB[:, :], VA.mult)
            nc.vector.tensor_tensor(WL[:, :], WL[:, :], t1[:, :], VA.add)
            nc.vector.tensor_tensor(WR[:, :], wx[1][:, :], eqA[:, :], VA.mult)
            nc.vector.tensor_tensor(t1[:, :], wx[0][:, :], eqC[:, :], VA.mult)
            nc.vector.tensor_tensor(WR[:, :], WR[:, :], t1[:, :], VA.add)
            # weights per (a, side): wcor2[a*2+side]
            wcor2 = []
            for a in range(2):
                for sd, Wside in ((0, WL), (1, WR)):
                    wc, _ = f3(f"wc{a}{sd}")
                    nc.vector.tensor_tensor(wc[:, :], wy[a][:, :], Wside[:, :], VA.mult)
                    wcor2.append(wc)
            # pair row indices idx = cy*64 + bx; fidx col = k*64+g*32+a*16+cl
            cys = []
            for a in range(2):
                cs, _ = f3(f"cys{a}")
                nc.vector.tensor_scalar_mul(cs[:, :], cy[a][:, :], float(W))
                cys.append(cs)
            fidx = fld.tile([128, 2 * FDIM], F32, name="fidx")
            fidx_r = fidx[:, :].rearrange("p (k g a cl) -> p a g cl k",
                                          k=K2, g=NG, a=2, cl=CLG)
            for a in range(2):
                nc.vector.tensor_tensor(fidx_r[:, a],
                                        cys[a][:, :].rearrange(
                                            "p (g cl k) -> p g cl k",
                                            g=NG, cl=CLG, k=K2),
                                        bx[:, :].rearrange(
                                            "p (g cl k) -> p g cl k",
                                            g=NG, cl=CLG, k=K2), VA.add)
            fidxi = fld.tile([128, 2 * FDIM], I16, name="fidxi")
            nc.vector.tensor_copy(fidxi[:, :], fidx[:, :])

            # ---- fold indices into SWDGE wrapped layout ----
            # idxw col = k*1024 + g*512 + m*128 + cl*8 + f ; value stream for
            # (k,g): i = m*2048 + cl*128 + 16f + p'  ->  (i%16, i//16)
            idxw = fld.tile([128, K2 * NG * 2 * CLG * 8], I16, name="idxw")
            src_r = fidxi[:, :].rearrange("p (k gacl) -> p k gacl",
                                          k=K2, gacl=64)
            dst_r = idxw[:, :].rearrange("p (k gacl f) -> p f k gacl",
                                         k=K2, gacl=64, f=8)
            for f in range(8):
                nc.sync.dma_start(dst_r[0:16, f],
                                  src_r[16 * f:16 * (f + 1)])
            for f in range(1, 8):
                nc.sync.dma_start(idxw[16 * f:16 * (f + 1), :], idxw[0:16, :])

            # ---- per-tap 1x1 convs:  yT[pix, (k,o)] = x_chunk^T @ wmain ----
            for c in range(NCH):
                lhs = xin_sb[:, c * 128:(c + 1) * 128]
                y_sb = sb.tile([128, K2 * O], BF16, name=f"ysb{c}", tag="ysb")
                for j in range(3):
                    y_ps = psp.tile([128, 384], F32, name=f"yps{c}_{j}", tag="yps")
                    nc.tensor.matmul(y_ps[:, :], lhs,
                                     wmain_sb[:, j * 384:(j + 1) * 384],
                                     start=True, stop=True)
                    if ycopy_act or c % 2 == 0:
                        nc.scalar.copy(y_sb[:, j * 384:(j + 1) * 384], y_ps[:, :])
                    else:
                        nc.vector.tensor_copy(y_sb[:, j * 384:(j + 1) * 384], y_ps[:, :])
                nc.sync.dma_start(
                    y_dram[:, c * 128:(c + 1) * 128, :].rearrange("k p o -> p k o"),
                    y_sb[:, :].rearrange("p (k o) -> p k o", o=O))

            # ---- gather + weighted accumulate ----
            for g in range(NG):
                psob = []
                for q in range(CLG // 4):
                    p = psop.tile([128, 512], F32, name=f"pso{g}_{q}", tag=f"pso{q}")
                    psob.append(p)
                psout = [psob[cl // 4][:, (cl % 4) * O:(cl % 4 + 1) * O]
                         for cl in range(CLG)]
                for k in range(K2):
                    ysrc = y_dram[k, :, :]
                    ypairs = bass.AP(tensor=ysrc.tensor, offset=ysrc.offset,
                                     ap=[[O, HW - 1], [1, 2 * O]])
                    for a in range(2):
                        gt = gth.tile([128, CLG, 2 * O], BF16,
                                      name=f"g{g}_{k}_{a}", tag="gath")
                        base = k * 512 + g * 256 + a * 128
                        for s in range(2):  # <=1024 descriptors per SWDGE inst
                            nc.gpsimd.dma_gather(
                                gt[:, s * 8:(s + 1) * 8, :], ypairs,
                                idxw[:, base + s * 64: base + (s + 1) * 64],
                                1024, 1024, 2 * O, elem_step=O)
                        for sd in range(2):
                            for q in range(CLG // 4):
                                tmp = tmppool.tile([128, 512], BF16,
                                                   name=f"t{g}_{k}_{a}_{sd}_{q}",
                                                   tag="tmp")
                                blk = (k * 8 + a * 4 + sd * 2 + (q & 1)) % act_mod if act_mod else 1
                                for j in range(4):
                                    cl = q * 4 + j
                                    c = g * CLG + cl
                                    if blk == 0:
                                        nc.scalar.activation(
                                            tmp[:, j * O:(j + 1) * O],
                                            gt[:, cl, sd * O:(sd + 1) * O],
                                            mybir.ActivationFunctionType.Copy,
                                            scale=wcor2[a * 2 + sd][:, c * K2 + k:
                                                                    c * K2 + k + 1])
                                    else:
                                        nc.vector.tensor_scalar_mul(
                                            tmp[:, j * O:(j + 1) * O],
                                            gt[:, cl, sd * O:(sd + 1) * O],
                                            wcor2[a * 2 + sd][:, c * K2 + k:
                                                              c * K2 + k + 1])
                                nc.tensor.matmul(psob[q][:, :], identb[:, :],
                                                 tmp[:, :],
                                                 start=(k == 0 and a == 0 and sd == 0),
                                                 stop=(k == K2 - 1 and a == 1 and sd == 1))
                for cl in range(CLG):
                    c = g * CLG + cl
                    ot = sb.tile([128, O], F32, name=f"o{g}_{cl}", tag="ot")
                    nc.vector.tensor_tensor(ot[:, :], psout[cl],
                                            bias_sb[:, :], mybir.AluOpType.add)
                    nc.sync.dma_start(out_dram[c * 128:(c + 1) * 128, :], ot[:, :])

    nc.compile()
    _split_excess_waits(nc)
    return nc


_NC_CACHE = None


def _get_nc():
    global _NC_CACHE
    if _NC_CACHE is None:
        _NC_CACHE = build_nc()
    return _NC_CACHE


def _host_inputs(x, offset_w, offset_b, weight, bias):
    bf = ml_dtypes.bfloat16
    # constant (shared) tensors
    offw = np.ascontiguousarray(
        offset_w.reshape(18, C, K2).transpose(1, 2, 0).reshape(C, K2 * 18)).astype(bf)
    wmain = np.ascontiguousarray(
        weight.reshape(O, C, K2).transpose(1, 2, 0).reshape(C, K2 * O)).astype(bf)
    offb = offset_b.reshape(18, 1).astype(np.float32)
    bias_t = np.tile(bias.reshape(1, O), (128, 1)).astype(np.float32)
    pi = np.arange(128)
    cc = np.arange(NCH)
    kk = np.arange(K2)
    pix = cc[None, :, None] * 128 + pi[:, None, None]          # [128, 32, 1]
    ybase = (pix // W - 1 + (kk // 3)[None, None, :]).reshape(128, FDIM_np).astype(np.float32)
    xbase = (pix % W - 1 + (kk % 3)[None, None, :]).reshape(128, FDIM_np).astype(np.float32)
    identf = np.eye(128, dtype=np.float32)
    identb = np.eye(128, dtype=bf)
    shared = dict(offw=offw, wmain=wmain, offb=offb, bias_t=bias_t,
                  ybase=ybase, xbase=xbase, identf=identf, identb=identb)
    maps = []
    for b in range(B):
        m = dict(shared)
        m["x_img"] = np.ascontiguousarray(x[b].reshape(C, HW)).astype(bf)
        maps.append(m)
    return maps


FDIM_np = NCH * K2


def kernel(x, offset_w, offset_b, weight, bias):
    from concourse.bass_utils import run_bass_kernel_spmd
    nc = _get_nc()
    in_maps = _host_inputs(np.asarray(x, np.float32), np.asarray(offset_w, np.float32),
                           np.asarray(offset_b, np.float32),
                           np.asarray(weight, np.float32), np.asarray(bias, np.float32))
    res = run_bass_kernel_spmd(nc, in_maps, core_ids=list(range(B)))
    out = np.stack([np.asarray(res.results[b]["out"], np.float32).T.reshape(O, H, W)
                    for b in range(B)])
    return out


# revision 24
# speedup vs baseline: 3186.0219x; 1.0149x over previous
"""Deformable conv net kernel for 8 TRN2 NeuronCores (data-parallel over batch).

Algorithm (per core, one batch sample):
  1. offsets = conv3x3(x, offset_w) + offset_b            (PE, bf16)
  2. per-pixel bilinear fields: corner indices + weights  (DVE, fp32)
  3. Y_k = W_k^T @ x for each of 9 taps (1x1 convs)       (PE, bf16)
     -- bilinear sampling commutes with the per-pixel 1x1 contraction,
        so we matmul first and gather afterwards.
  4. gather Y_k rows at the 4 corner indices              (SWDGE dma_gather)
  5. out[pix, o] = sum_{k,m} w_{k,m}[pix] * gath[pix, o]  (DVE tensor_scalar
     + PE identity-matmul accumulation into PSUM)
  6. out += bias; host reassembles [8, 128, 64, 64].
"""
import os, sys

for _p in ("/opt/trn_rl_repo", "/root/.axon_site/_ro/trn_rl_repo"):
    if os.path.isdir(_p) and _p not in sys.path:
        sys.path.insert(0, _p)

import numpy as np
import ml_dtypes

import concourse.bass as bass
import concourse.mybir as mybir
from concourse import bacc, library_config
from concourse.tile import TileContext

BF16 = mybir.dt.bfloat16
F32 = mybir.dt.float32
I16 = mybir.dt.int16

B, C, H, W = 8, 128, 64, 64
O = 128
K = 3
K2 = 9
HW = H * W                 # 4096
NCH = HW // 128            # 32 pixel chunks of 128
NG = 2                     # pixel groups for the gather phase
CLG = NCH // NG            # 16 chunks per group
MAGIC = float(3 * 2 ** 22)  # 1.5*2^23: keeps s+M in the ulp=1 binade

_MAX_WAITS = 1             # this walrus build rejects >1 sem wait per inst


def _split_excess_waits(nc):
    for f in nc.m.functions:
        for bb in f.blocks:
            new_insts = []
            for inst in bb.instructions:
                si = inst.sync_info
                if si is not None and si.on_wait and len(si.on_wait) > _MAX_WAITS:
                    waits = list(si.on_wait)
                    keep = waits[-_MAX_WAITS:]
                    spill = waits[:-_MAX_WAITS]
                    for j in range(0, len(spill), _MAX_WAITS):
                        chunk = spill[j:j + _MAX_WAITS]
                        nop = mybir.InstNoOp(
                            name=f"{inst.name}-wsp{j}",
                            engine=inst.engine,
                            ins=[], outs=[],
                            sync_info=mybir.SyncInfo(on_wait=chunk, on_update=[]),
                        )
                        nc.register_instruction(nop, overwrite=True)
                        new_insts.append(nop)
                    inst.sync_info = mybir.SyncInfo(
                        on_wait=keep, on_update=list(si.on_update or []))
                new_insts.append(inst)
            bb.instructions[:] = new_insts


def build_nc(act_mod=0, ycopy_act=True):
    nc = bacc.Bacc()
    x_in = nc.dram_tensor("x_img", [C, HW], BF16, kind="ExternalInput")
    offw_in = nc.dram_tensor("offw", [C, K2 * 18], BF16, kind="ExternalInput")
    offb_in = nc.dram_tensor("offb", [18, 1], F32, kind="ExternalInput")
    wmain_in = nc.dram_tensor("wmain", [C, K2 * O], BF16, kind="ExternalInput")
    bias_in = nc.dram_tensor("bias_t", [128, O], F32, kind="ExternalInput")
    ybase_in = nc.dram_tensor("ybase", [128, NCH * K2], F32, kind="ExternalInput")
    xbase_in = nc.dram_tensor("xbase", [128, NCH * K2], F32, kind="ExternalInput")
    idf_in = nc.dram_tensor("identf", [128, 128], F32, kind="ExternalInput")
    idb_in = nc.dram_tensor("identb", [128, 128], BF16, kind="ExternalInput")
    y_dram = nc.dram_tensor("y_scratch", [K2, HW, O], BF16)
    out_dram = nc.dram_tensor("out", [HW, O], F32, kind="ExternalOutput")

    FDIM = NCH * K2  # 288, (c, k) col = c*9 + k

    with TileContext(nc) as tc:
        with tc.tile_pool(name="cst", bufs=1) as cst, \
             tc.tile_pool(name="sb", bufs=2) as sb, \
             tc.tile_pool(name="fld", bufs=1) as fld, \
             tc.tile_pool(name="gth", bufs=3) as gth, \
             tc.tile_pool(name="tmp", bufs=4) as tmppool, \
             tc.tile_pool(name="ps", bufs=2, space="PSUM") as psp, \
             tc.tile_pool(name="pso", bufs=1, space="PSUM") as psop:

            nc.gpsimd.load_library(library_config.mlp)

            # Tiny SWDGE op up front: bass barriers POOL's first dynamic DMA
            # against ALL outstanding HWDGE lanes; firing it now (nothing in
            # flight) keeps that barrier off the gather critical path.
            warm = cst.tile([16, 16], BF16, name="warm")
            nc.gpsimd.dma_start(warm[:, :], x_in[0:16, 0:16])

            # ---- constant / input loads ----
            offw_sb = cst.tile([C, K2 * 18], BF16, name="offw_sb")
            nc.sync.dma_start(offw_sb[:, :], offw_in[:, :])
            wmain_sb = cst.tile([C, K2 * O], BF16, name="wmain_sb")
            nc.sync.dma_start(wmain_sb[:, :], wmain_in[:, :])
            offb_sb = cst.tile([18, 1], F32, name="offb_sb")
            nc.sync.dma_start(offb_sb[:, :], offb_in[:, :])
            bias_sb = cst.tile([128, O], F32, name="bias_sb")
            nc.sync.dma_start(bias_sb[:, :], bias_in[:, :])
            ybase_sb = cst.tile([128, FDIM], F32, name="ybase_sb")
            nc.sync.dma_start(ybase_sb[:, :], ybase_in[:, :])
            xbase_sb = cst.tile([128, FDIM], F32, name="xbase_sb")
            nc.sync.dma_start(xbase_sb[:, :], xbase_in[:, :])
            identf = cst.tile([128, 128], F32, name="identf")
            nc.sync.dma_start(identf[:, :], idf_in[:, :])
            identb = cst.tile([128, 128], BF16, name="identb")
            nc.sync.dma_start(identb[:, :], idb_in[:, :])

            # ---- contiguous input (for 1x1-conv lhsT) ----
            xin_sb = cst.tile([C, HW], BF16, name="xin_sb")
            nc.sync.dma_start(xin_sb[:, :], x_in[:, :])

            # ---- row-padded image with 1-elem guards (contiguous conv rhs) ----
            XPR = (H + 4) * W  # extra zero rows double as guards
            xpr = cst.tile([C, XPR], BF16, name="xpr")
            nc.vector.memset(xpr[:, :], 0.0)
            nc.sync.dma_start(xpr[:, 1 + W: 1 + W + HW], x_in[:, :])

            # ---- offset conv: offsets [18, HW] fp32 ----
            off_sb = fld.tile([18, HW], F32, name="off_sb")
            # contiguous N=512 conv with x-wrap, corrected at columns 0/63
            corr_ps = psp.tile([18, 2 * H], F32, name="corr_ps", tag="ph1ps")
            colL = xpr[:, 0:(H + 2) * W].rearrange("c (r w) -> c w r", w=W)
            colR = xpr[:, 1:1 + (H + 3) * W].rearrange("c (r w) -> c w r", w=W)
            for kh in range(3):
                nc.tensor.matmul(corr_ps[:, 0:H],
                                 offw_sb[:, (3 * kh) * 18:(3 * kh + 1) * 18],
                                 colL[:, 0, kh:kh + H],
                                 start=(kh == 0), stop=(kh == 2))
            for kh in range(3):
                nc.tensor.matmul(corr_ps[:, H:2 * H],
                                 offw_sb[:, (3 * kh + 2) * 18:(3 * kh + 3) * 18],
                                 colR[:, 0, kh + 1:kh + 1 + H],
                                 start=(kh == 0), stop=(kh == 2))
            for n in range(8):
                off_ps = psp.tile([18, 512], F32, name=f"offps{n}", tag="ph1ps")
                for k in range(K2):
                    kh, kw = k // 3, k % 3
                    base = 1 + (n * 8 + kh) * W + (kw - 1)
                    nc.tensor.matmul(off_ps[:, :], offw_sb[:, k * 18:(k + 1) * 18],
                                     xpr[:, base: base + 512],
                                     start=(k == 0), stop=(k == K2 - 1))
                nc.vector.tensor_scalar_add(off_sb[:, n * 512:(n + 1) * 512],
                                            off_ps[:, :], offb_sb[:, 0:1])
            offv = off_sb[:, :].rearrange("j (y x) -> j y x", x=W)
            nc.vector.tensor_tensor(
                offv[:, :, 0:1].rearrange("j y one -> j (y one)"),
                offv[:, :, 0:1].rearrange("j y one -> j (y one)"),
                corr_ps[:, 0:H], mybir.AluOpType.subtract)
            nc.vector.tensor_tensor(
                offv[:, :, W - 1:W].rearrange("j y one -> j (y one)"),
                offv[:, :, W - 1:W].rearrange("j y one -> j (y one)"),
                corr_ps[:, H:2 * H], mybir.AluOpType.subtract)

            # ---- transpose offsets to pixel-major: offT [128, 32*18] ----
            offT = fld.tile([128, NCH * 18], F32, name="offT")
            for c in range(NCH):
                tr_ps = psp.tile([128, 18], F32, name=f"trps{c}", tag="ph1ps")
                nc.tensor.transpose(tr_ps[:, :], off_sb[:, c * 128:(c + 1) * 128],
                                    identf[:18, :18])
                nc.scalar.copy(offT[:, c * 18:(c + 1) * 18], tr_ps[:, :])

            psp_cm.__exit__(None, None, None)
            # ---- bilinear fields (fp32, [128, (c,k)=288]) ----
            offT4 = offT[:, :].rearrange("p (c k two) -> p two c k", two=2, k=K2)
            yb3 = ybase_sb[:, :].rearrange("p (c k) -> p c k", k=K2)
            xb3 = xbase_sb[:, :].rearrange("p (c k) -> p c k", k=K2)

            def f3(name):
                t = fld.tile([128, FDIM], F32, name=name, tag=name)
                return t, t[:, :].rearrange("p (c k) -> p c k", k=K2)

            VA = mybir.AluOpType
            axes = {}
            axes_i0 = {}
            for ax in ("y", "x"):
                s, s3 = f3(f"s_{ax}")
                base3 = yb3 if ax == "y" else xb3
                nc.vector.tensor_tensor(s3, offT4[:, 0 if ax == "y" else 1], base3, VA.add)
                r, r3 = f3(f"r_{ax}")
                nc.vector.tensor_scalar_add(r[:, :], s[:, :], MAGIC)
                nc.vector.tensor_scalar_add(r[:, :], r[:, :], -MAGIC)
                g, g3 = f3(f"g_{ax}")
                nc.vector.tensor_tensor(g[:, :], r[:, :], s[:, :], VA.is_gt)
                i0, _ = f3(f"i0_{ax}")
                nc.vector.tensor_tensor(i0[:, :], r[:, :], g[:, :], VA.subtract)
                fr, _ = f3(f"fr_{ax}")
                nc.vector.tensor_tensor(fr[:, :], s[:, :], i0[:, :], VA.subtract)
                i1, _ = f3(f"i1_{ax}")
                nc.vector.tensor_scalar_add(i1[:, :], i0[:, :], 1.0)
                w_m = []
                for (ii, frac_is_w) in ((i0, False), (i1, True)):
                    v, _ = f3(f"v_{ax}_{frac_is_w}")
                    nc.vector.tensor_scalar(v[:, :], ii[:, :], 0.0, None, VA.is_ge)
                    t2, _ = f3(f"t2_{ax}_{frac_is_w}")
                    nc.vector.tensor_scalar(t2[:, :], ii[:, :], float(H - 1), None, VA.is_le)
                    nc.vector.tensor_tensor(v[:, :], v[:, :], t2[:, :], VA.mult)
                    wm, _ = f3(f"wm_{ax}_{frac_is_w}")
                    if frac_is_w:
                        nc.vector.tensor_tensor(wm[:, :], fr[:, :], v[:, :], VA.mult)
                    else:
                        nc.vector.tensor_scalar(wm[:, :], fr[:, :], -1.0, 1.0,
                                                VA.mult, VA.add)
                        nc.vector.tensor_tensor(wm[:, :], wm[:, :], v[:, :], VA.mult)
                    w_m.append(wm)
                cl = []
                for ii in (i0, i1):
                    cc, _ = f3(f"c_{ax}_{ii is i1}")
                    nc.vector.tensor_scalar(cc[:, :], ii[:, :], 0.0, float(H - 1),
                                            VA.max, VA.min)
                    cl.append(cc)
                axes[ax] = (w_m, cl)
                axes_i0[ax] = i0

            (wy, cy), (wx, _cxunused) = axes["y"], axes["x"]
            # pair-fetch base bx = clip(ix0, 0, 62); weights for pair slots
            ix0f = axes_i0["x"]
            bx, _ = f3("bx")
            nc.vector.tensor_scalar(bx[:, :], ix0f[:, :], 0.0, float(W - 2),
                                    VA.max, VA.min)
            dif, _ = f3("dif")
            nc.vector.tensor_tensor(dif[:, :], bx[:, :], ix0f[:, :], VA.subtract)
            eqA, _ = f3("eqA")
            nc.vector.tensor_scalar(eqA[:, :], dif[:, :], 0.0, None, VA.is_equal)
            eqB, _ = f3("eqB")
            nc.vector.tensor_scalar(eqB[:, :], dif[:, :], 1.0, None, VA.is_equal)
            eqC, _ = f3("eqC")
            nc.vector.tensor_scalar(eqC[:, :], dif[:, :], -1.0, None, VA.is_equal)
            WL, _ = f3("WL")
            WR, _ = f3("WR")
            t1, _ = f3("t1")
            nc.vector.tensor_tensor(WL[:, :], wx[0][:, :], eqA[:, :], VA.mult)
            nc.vector.tensor_tensor(t1[:, :], wx[1][:, :], eqB[:, :], VA.mult)
            nc.vector.tensor_tensor(WL[:, :], WL[:, :], t1[:, :], VA.add)
            nc.vector.tensor_tensor(WR[:, :], wx[1][:, :], eqA[:, :], VA.mult)
            nc.vector.tensor_tensor(t1[:, :], wx[0][:, :], eqC[:, :], VA.mult)
            nc.vector.tensor_tensor(WR[:, :], WR[:, :], t1[:, :], VA.add)
            # weights per (a, side): wcor2[a*2+side]
            wcor2 = []
            for a in range(2):
                for sd, Wside in ((0, WL), (1, WR)):
                    wc, _ = f3(f"wc{a}{sd}")
                    nc.vector.tensor_tensor(wc[:, :], wy[a][:, :], Wside[:, :], VA.mult)
                    wcor2.append(wc)
            # pair row indices idx = cy*64 + bx; fidx col = k*64+g*32+a*16+cl
            cys = []
            for a in range(2):
                cs, _ = f3(f"cys{a}")
                nc.vector.tensor_scalar_mul(cs[:, :], cy[a][:, :], float(W))
                cys.append(cs)
            fidx = fld.tile([128, 2 * FDIM], F32, name="fidx")
            fidx_r = fidx[:, :].rearrange("p (k g a cl) -> p a g cl k",
                                          k=K2, g=NG, a=2, cl=CLG)
            for a in range(2):
                nc.vector.tensor_tensor(fidx_r[:, a],
                                        cys[a][:, :].rearrange(
                                            "p (g cl k) -> p g cl k",
                                            g=NG, cl=CLG, k=K2),
                                        bx[:, :].rearrange(
                                            "p (g cl k) -> p g cl k",
                                            g=NG, cl=CLG, k=K2), VA.add)
            fidxi = fld.tile([128, 2 * FDIM], I16, name="fidxi")
            nc.vector.tensor_copy(fidxi[:, :], fidx[:, :])

            # ---- fold indices into SWDGE wrapped layout ----
            # idxw col = k*1024 + g*512 + m*128 + cl*8 + f ; value stream for
            # (k,g): i = m*2048 + cl*128 + 16f + p'  ->  (i%16, i//16)
            idxw = fld.tile([128, K2 * NG * 2 * CLG * 8], I16, name="idxw")
            src_r = fidxi[:, :].rearrange("p (k gacl) -> p k gacl",
                                          k=K2, gacl=64)
            dst_r = idxw[:, :].rearrange("p (k gacl f) -> p f k gacl",
                                         k=K2, gacl=64, f=8)
            # ACT HWDGE ring: keeps these off the SP FIFO, which is
            # head-of-line blocked by y-write DMAs waiting on ACT copies.
            for f in range(8):
                nc.scalar.dma_start(dst_r[0:16, f],
                                    src_r[16 * f:16 * (f + 1)])
            for f in range(1, 8):
                nc.scalar.dma_start(idxw[16 * f:16 * (f + 1), :], idxw[0:16, :])

            psp_cm = tc.tile_pool(name="ps", bufs=2, space="PSUM")
            psp = psp_cm.__enter__()
            # ---- per-tap 1x1 convs:  yT[pix, (k,o)] = x_chunk^T @ wmain ----
            for c in range(NCH):
                lhs = xin_sb[:, c * 128:(c + 1) * 128]
                y_sb = sb.tile([128, K2 * O], BF16, name=f"ysb{c}", tag="ysb")
                for j in range(3):
                    y_ps = psp.tile([128, 384], F32, name=f"yps{c}_{j}", tag="yps")
                    nc.tensor.matmul(y_ps[:, :], lhs,
                                     wmain_sb[:, j * 384:(j + 1) * 384],
                                     start=True, stop=True)
                    if ycopy_act or c % 2 == 0:
                        nc.scalar.copy(y_sb[:, j * 384:(j + 1) * 384], y_ps[:, :])
                    else:
                        nc.vector.tensor_copy(y_sb[:, j * 384:(j + 1) * 384], y_ps[:, :])
                nc.sync.dma_start(
                    y_dram[:, c * 128:(c + 1) * 128, :].rearrange("k p o -> p k o"),
                    y_sb[:, :].rearrange("p (k o) -> p k o", o=O))

            # ---- gather + weighted accumulate ----
            psop_cm = tc.tile_pool(name="pso", bufs=1, space="PSUM")
            psop = psop_cm.__enter__()
            for g in range(NG):
                psob = []
                for q in range(CLG // 4):
                    p = psop.tile([128, 512], F32, name=f"pso{g}_{q}", tag=f"pso{g}_{q}")
                    psob.append(p)
                psout = [psob[cl // 4][:, (cl % 4) * O:(cl % 4 + 1) * O]
                         for cl in range(CLG)]
                for k in range(K2):
                    ysrc = y_dram[k, :, :]
                    ypairs = bass.AP(tensor=ysrc.tensor, offset=ysrc.offset,
                                     ap=[[O, HW - 1], [1, 2 * O]])
                    for a in range(2):
                        gt = gth.tile([128, CLG, 2 * O], BF16,
                                      name=f"g{g}_{k}_{a}", tag="gath")
                        base = k * 512 + g * 256 + a * 128
                        for s in range(2):  # <=1024 descriptors per SWDGE inst
                            nc.gpsimd.dma_gather(
                                gt[:, s * 8:(s + 1) * 8, :], ypairs,
                                idxw[:, base + s * 64: base + (s + 1) * 64],
                                1024, 1024, 2 * O, elem_step=O)
                        for sd in range(2):
                            for q in range(CLG // 4):
                                tmp = tmppool.tile([128, 512], BF16,
                                                   name=f"t{g}_{k}_{a}_{sd}_{q}",
                                                   tag="tmp")
                                blk = (k * 8 + a * 4 + sd * 2 + (q & 1)) % act_mod if act_mod else 1
                                for j in range(4):
                                    cl = q * 4 + j
                                    c = g * CLG + cl
                                    if blk == 0:
                                        nc.scalar.activation(
                                            tmp[:, j * O:(j + 1) * O],
                                            gt[:, cl, sd * O:(sd + 1) * O],
                                            mybir.ActivationFunctionType.Copy,
                                            scale=wcor2[a * 2 + sd][:, c * K2 + k:
                                                                    c * K2 + k + 1])
                                    else:
                                        nc.vector.tensor_scalar_mul(
                                            tmp[:, j * O:(j + 1) * O],
                                            gt[:, cl, sd * O:(sd + 1) * O],
                                            wcor2[a * 2 + sd][:, c * K2 + k:
                                                              c * K2 + k + 1])
                                nc.tensor.matmul(psob[q][:, :], identb[:, :],
                                                 tmp[:, :],
                                                 start=(k == 0 and a == 0 and sd == 0),
                                                 stop=(k == K2 - 1 and a == 1 and sd == 1))
                for cl in range(CLG):
                    c = g * CLG + cl
                    ot = sb.tile([128, O], F32, name=f"o{g}_{cl}", tag="ot")
                    nc.vector.tensor_tensor(ot[:, :], psout[cl],
                                            bias_sb[:, :], mybir.AluOpType.add)
                    nc.sync.dma_start(out_dram[c * 128:(c + 1) * 128, :], ot[:, :])
            psop_cm.__exit__(None, None, None)

    nc.compile()
    _split_excess_waits(nc)
    return nc


_NC_CACHE = None


def _get_nc():
    global _NC_CACHE
    if _NC_CACHE is None:
        _NC_CACHE = build_nc()
    return _NC_CACHE


def _host_inputs(x, offset_w, offset_b, weight, bias):
    bf = ml_dtypes.bfloat16
    # constant (shared) tensors
    offw = np.ascontiguousarray(
        offset_w.reshape(18, C, K2).transpose(1, 2, 0).reshape(C, K2 * 18)).astype(bf)
    wmain = np.ascontiguousarray(
        weight.reshape(O, C, K2).transpose(1, 2, 0).reshape(C, K2 * O)).astype(bf)
    offb = offset_b.reshape(18, 1).astype(np.float32)
    bias_t = np.tile(bias.reshape(1, O), (128, 1)).astype(np.float32)
    pi = np.arange(128)
    cc = np.arange(NCH)
    kk = np.arange(K2)
    pix = cc[None, :, None] * 128 + pi[:, None, None]          # [128, 32, 1]
    ybase = (pix // W - 1 + (kk // 3)[None, None, :]).reshape(128, FDIM_np).astype(np.float32)
    xbase = (pix % W - 1 + (kk % 3)[None, None, :]).reshape(128, FDIM_np).astype(np.float32)
    identf = np.eye(128, dtype=np.float32)
    identb = np.eye(128, dtype=bf)
    shared = dict(offw=offw, wmain=wmain, offb=offb, bias_t=bias_t,
                  ybase=ybase, xbase=xbase, identf=identf, identb=identb)
    maps = []
    for b in range(B):
        m = dict(shared)
        m["x_img"] = np.ascontiguousarray(x[b].reshape(C, HW)).astype(bf)
        maps.append(m)
    return maps


FDIM_np = NCH * K2


def kernel(x, offset_w, offset_b, weight, bias):
    from concourse.bass_utils import run_bass_kernel_spmd
    nc = _get_nc()
    in_maps = _host_inputs(np.asarray(x, np.float32), np.asarray(offset_w, np.float32),
                           np.asarray(offset_b, np.float32),
                           np.asarray(weight, np.float32), np.asarray(bias, np.float32))
    res = run_bass_kernel_spmd(nc, in_maps, core_ids=list(range(B)))
    out = np.stack([np.asarray(res.results[b]["out"], np.float32).T.reshape(O, H, W)
                    for b in range(B)])
    return out


# revision 25
# speedup vs baseline: 3301.1381x; 1.0361x over previous
"""Deformable conv net kernel for 8 TRN2 NeuronCores (data-parallel over batch).

Algorithm (per core, one batch sample):
  1. offsets = conv3x3(x, offset_w) + offset_b            (PE, bf16)
  2. per-pixel bilinear fields: corner indices + weights  (DVE, fp32)
  3. Y_k = W_k^T @ x for each of 9 taps (1x1 convs)       (PE, bf16)
     -- bilinear sampling commutes with the per-pixel 1x1 contraction,
        so we matmul first and gather afterwards.
  4. gather Y_k rows at the 4 corner indices              (SWDGE dma_gather)
  5. out[pix, o] = sum_{k,m} w_{k,m}[pix] * gath[pix, o]  (DVE tensor_scalar
     + PE identity-matmul accumulation into PSUM)
  6. out += bias; host reassembles [8, 128, 64, 64].
"""
import os, sys

for _p in ("/opt/trn_rl_repo", "/root/.axon_site/_ro/trn_rl_repo"):
    if os.path.isdir(_p) and _p not in sys.path:
        sys.path.insert(0, _p)

import numpy as np
import ml_dtypes

import concourse.bass as bass
import concourse.mybir as mybir
from concourse import bacc, library_config
from concourse.tile import TileContext

BF16 = mybir.dt.bfloat16
F32 = mybir.dt.float32
I16 = mybir.dt.int16

B, C, H, W = 8, 128, 64, 64
O = 128
K = 3
K2 = 9
HW = H * W                 # 4096
NCH = HW // 128            # 32 pixel chunks of 128
NG = 4                     # pixel groups for the gather phase
CLG = NCH // NG            # 16 chunks per group
MAGIC = float(3 * 2 ** 22)  # 1.5*2^23: keeps s+M in the ulp=1 binade

_MAX_WAITS = 1             # this walrus build rejects >1 sem wait per inst


def _split_excess_waits(nc):
    for f in nc.m.functions:
        for bb in f.blocks:
            new_insts = []
            for inst in bb.instructions:
                si = inst.sync_info
                if si is not None and si.on_wait and len(si.on_wait) > _MAX_WAITS:
                    waits = list(si.on_wait)
                    keep = waits[-_MAX_WAITS:]
                    spill = waits[:-_MAX_WAITS]
                    for j in range(0, len(spill), _MAX_WAITS):
                        chunk = spill[j:j + _MAX_WAITS]
                        nop = mybir.InstNoOp(
                            name=f"{inst.name}-wsp{j}",
                            engine=inst.engine,
                            ins=[], outs=[],
                            sync_info=mybir.SyncInfo(on_wait=chunk, on_update=[]),
                        )
                        nc.register_instruction(nop, overwrite=True)
                        new_insts.append(nop)
                    inst.sync_info = mybir.SyncInfo(
                        on_wait=keep, on_update=list(si.on_update or []))
                new_insts.append(inst)
            bb.instructions[:] = new_insts


def build_nc(act_mod=0, ycopy_act=True):
    nc = bacc.Bacc()
    x_in = nc.dram_tensor("x_img", [C, HW], BF16, kind="ExternalInput")
    offw_in = nc.dram_tensor("offw", [C, K2 * 18], BF16, kind="ExternalInput")
    offb_in = nc.dram_tensor("offb", [18, 1], F32, kind="ExternalInput")
    wmain_in = nc.dram_tensor("wmain", [C, K2 * O], BF16, kind="ExternalInput")
    bias_in = nc.dram_tensor("bias_t", [128, O], F32, kind="ExternalInput")
    ybase_in = nc.dram_tensor("ybase", [128, NCH * K2], F32, kind="ExternalInput")
    xbase_in = nc.dram_tensor("xbase", [128, NCH * K2], F32, kind="ExternalInput")
    idf_in = nc.dram_tensor("identf", [128, 128], F32, kind="ExternalInput")
    idb_in = nc.dram_tensor("identb", [128, 128], BF16, kind="ExternalInput")
    y_dram = nc.dram_tensor("y_scratch", [K2, HW, O], BF16)
    out_dram = nc.dram_tensor("out", [HW, O], F32, kind="ExternalOutput")

    FDIM = NCH * K2  # 288, (c, k) col = c*9 + k

    with TileContext(nc) as tc:
        with tc.tile_pool(name="cst", bufs=1) as cst, \
             tc.tile_pool(name="sb", bufs=2) as sb, \
             tc.tile_pool(name="fld", bufs=1) as fld, \
             tc.tile_pool(name="gth", bufs=3) as gth, \
             tc.tile_pool(name="tmp", bufs=4) as tmppool, \
             tc.tile_pool(name="ps", bufs=2, space="PSUM") as psp, \
             tc.tile_pool(name="pso", bufs=1, space="PSUM") as psop:

            nc.gpsimd.load_library(library_config.mlp)

            # Tiny SWDGE op up front: bass barriers POOL's first dynamic DMA
            # against ALL outstanding HWDGE lanes; firing it now (nothing in
            # flight) keeps that barrier off the gather critical path.
            warm = cst.tile([16, 16], BF16, name="warm")
            nc.gpsimd.dma_start(warm[:, :], x_in[0:16, 0:16])

            # ---- constant / input loads ----
            offw_sb = cst.tile([C, K2 * 18], BF16, name="offw_sb")
            nc.sync.dma_start(offw_sb[:, :], offw_in[:, :])
            wmain_sb = cst.tile([C, K2 * O], BF16, name="wmain_sb")
            nc.sync.dma_start(wmain_sb[:, :], wmain_in[:, :])
            offb_sb = cst.tile([18, 1], F32, name="offb_sb")
            nc.sync.dma_start(offb_sb[:, :], offb_in[:, :])
            bias_sb = cst.tile([128, O], F32, name="bias_sb")
            nc.sync.dma_start(bias_sb[:, :], bias_in[:, :])
            ybase_sb = cst.tile([128, FDIM], F32, name="ybase_sb")
            nc.sync.dma_start(ybase_sb[:, :], ybase_in[:, :])
            xbase_sb = cst.tile([128, FDIM], F32, name="xbase_sb")
            nc.sync.dma_start(xbase_sb[:, :], xbase_in[:, :])
            identf = cst.tile([128, 128], F32, name="identf")
            nc.sync.dma_start(identf[:, :], idf_in[:, :])
            identb = cst.tile([128, 128], BF16, name="identb")
            nc.sync.dma_start(identb[:, :], idb_in[:, :])

            # ---- contiguous input (for 1x1-conv lhsT) ----
            xin_sb = cst.tile([C, HW], BF16, name="xin_sb")
            nc.sync.dma_start(xin_sb[:, :], x_in[:, :])

            # ---- row-padded image with 1-elem guards (contiguous conv rhs) ----
            XPR = (H + 4) * W  # extra zero rows double as guards
            xpr = cst.tile([C, XPR], BF16, name="xpr")
            nc.vector.memset(xpr[:, :], 0.0)
            nc.sync.dma_start(xpr[:, 1 + W: 1 + W + HW], x_in[:, :])

            # ---- offset conv: offsets [18, HW] fp32 ----
            off_sb = fld.tile([18, HW], F32, name="off_sb")
            # contiguous N=512 conv with x-wrap, corrected at columns 0/63
            corr_ps = psp.tile([18, 2 * H], F32, name="corr_ps", tag="ph1ps")
            colL = xpr[:, 0:(H + 2) * W].rearrange("c (r w) -> c w r", w=W)
            colR = xpr[:, 1:1 + (H + 3) * W].rearrange("c (r w) -> c w r", w=W)
            for kh in range(3):
                nc.tensor.matmul(corr_ps[:, 0:H],
                                 offw_sb[:, (3 * kh) * 18:(3 * kh + 1) * 18],
                                 colL[:, 0, kh:kh + H],
                                 start=(kh == 0), stop=(kh == 2))
            for kh in range(3):
                nc.tensor.matmul(corr_ps[:, H:2 * H],
                                 offw_sb[:, (3 * kh + 2) * 18:(3 * kh + 3) * 18],
                                 colR[:, 0, kh + 1:kh + 1 + H],
                                 start=(kh == 0), stop=(kh == 2))
            for n in range(8):
                off_ps = psp.tile([18, 512], F32, name=f"offps{n}", tag="ph1ps")
                for k in range(K2):
                    kh, kw = k // 3, k % 3
                    base = 1 + (n * 8 + kh) * W + (kw - 1)
                    nc.tensor.matmul(off_ps[:, :], offw_sb[:, k * 18:(k + 1) * 18],
                                     xpr[:, base: base + 512],
                                     start=(k == 0), stop=(k == K2 - 1))
                nc.vector.tensor_scalar_add(off_sb[:, n * 512:(n + 1) * 512],
                                            off_ps[:, :], offb_sb[:, 0:1])
            offv = off_sb[:, :].rearrange("j (y x) -> j y x", x=W)
            nc.vector.tensor_tensor(
                offv[:, :, 0:1].rearrange("j y one -> j (y one)"),
                offv[:, :, 0:1].rearrange("j y one -> j (y one)"),
                corr_ps[:, 0:H], mybir.AluOpType.subtract)
            nc.vector.tensor_tensor(
                offv[:, :, W - 1:W].rearrange("j y one -> j (y one)"),
                offv[:, :, W - 1:W].rearrange("j y one -> j (y one)"),
                corr_ps[:, H:2 * H], mybir.AluOpType.subtract)

            # ---- transpose offsets to pixel-major: offT [128, 32*18] ----
            offT = fld.tile([128, NCH * 18], F32, name="offT")
            for c in range(NCH):
                tr_ps = psp.tile([128, 18], F32, name=f"trps{c}", tag="ph1ps")
                nc.tensor.transpose(tr_ps[:, :], off_sb[:, c * 128:(c + 1) * 128],
                                    identf[:18, :18])
                nc.scalar.copy(offT[:, c * 18:(c + 1) * 18], tr_ps[:, :])

            psp_cm.__exit__(None, None, None)
            # ---- bilinear fields (fp32, [128, (c,k)=288]) ----
            offT4 = offT[:, :].rearrange("p (c k two) -> p two c k", two=2, k=K2)
            yb3 = ybase_sb[:, :].rearrange("p (c k) -> p c k", k=K2)
            xb3 = xbase_sb[:, :].rearrange("p (c k) -> p c k", k=K2)

            def f3(name):
                t = fld.tile([128, FDIM], F32, name=name, tag=name)
                return t, t[:, :].rearrange("p (c k) -> p c k", k=K2)

            VA = mybir.AluOpType
            axes = {}
            axes_i0 = {}
            for ax in ("y", "x"):
                s, s3 = f3(f"s_{ax}")
                base3 = yb3 if ax == "y" else xb3
                nc.vector.tensor_tensor(s3, offT4[:, 0 if ax == "y" else 1], base3, VA.add)
                r, r3 = f3(f"r_{ax}")
                nc.vector.tensor_scalar_add(r[:, :], s[:, :], MAGIC)
                nc.vector.tensor_scalar_add(r[:, :], r[:, :], -MAGIC)
                g, g3 = f3(f"g_{ax}")
                nc.vector.tensor_tensor(g[:, :], r[:, :], s[:, :], VA.is_gt)
                i0, _ = f3(f"i0_{ax}")
                nc.vector.tensor_tensor(i0[:, :], r[:, :], g[:, :], VA.subtract)
                fr, _ = f3(f"fr_{ax}")
                nc.vector.tensor_tensor(fr[:, :], s[:, :], i0[:, :], VA.subtract)
                i1, _ = f3(f"i1_{ax}")
                nc.vector.tensor_scalar_add(i1[:, :], i0[:, :], 1.0)
                w_m = []
                for (ii, frac_is_w) in ((i0, False), (i1, True)):
                    v, _ = f3(f"v_{ax}_{frac_is_w}")
                    nc.vector.tensor_scalar(v[:, :], ii[:, :], 0.0, None, VA.is_ge)
                    t2, _ = f3(f"t2_{ax}_{frac_is_w}")
                    nc.vector.tensor_scalar(t2[:, :], ii[:, :], float(H - 1), None, VA.is_le)
                    nc.vector.tensor_tensor(v[:, :], v[:, :], t2[:, :], VA.mult)
                    wm, _ = f3(f"wm_{ax}_{frac_is_w}")
                    if frac_is_w:
                        nc.vector.tensor_tensor(wm[:, :], fr[:, :], v[:, :], VA.mult)
                    else:
                        nc.vector.tensor_scalar(wm[:, :], fr[:, :], -1.0, 1.0,
                                                VA.mult, VA.add)
                        nc.vector.tensor_tensor(wm[:, :], wm[:, :], v[:, :], VA.mult)
                    w_m.append(wm)
                cl = []
                for ii in (i0, i1):
                    cc, _ = f3(f"c_{ax}_{ii is i1}")
                    nc.vector.tensor_scalar(cc[:, :], ii[:, :], 0.0, float(H - 1),
                                            VA.max, VA.min)
                    cl.append(cc)
                axes[ax] = (w_m, cl)
                axes_i0[ax] = i0

            (wy, cy), (wx, _cxunused) = axes["y"], axes["x"]
            # pair-fetch base bx = clip(ix0, 0, 62); weights for pair slots
            ix0f = axes_i0["x"]
            bx, _ = f3("bx")
            nc.vector.tensor_scalar(bx[:, :], ix0f[:, :], 0.0, float(W - 2),
                                    VA.max, VA.min)
            dif, _ = f3("dif")
            nc.vector.tensor_tensor(dif[:, :], bx[:, :], ix0f[:, :], VA.subtract)
            eqA, _ = f3("eqA")
            nc.vector.tensor_scalar(eqA[:, :], dif[:, :], 0.0, None, VA.is_equal)
            eqB, _ = f3("eqB")
            nc.vector.tensor_scalar(eqB[:, :], dif[:, :], 1.0, None, VA.is_equal)
            eqC, _ = f3("eqC")
            nc.vector.tensor_scalar(eqC[:, :], dif[:, :], -1.0, None, VA.is_equal)
            WL, _ = f3("WL")
            WR, _ = f3("WR")
            t1, _ = f3("t1")
            nc.vector.tensor_tensor(WL[:, :], wx[0][:, :], eqA[:, :], VA.mult)
            nc.vector.tensor_tensor(t1[:, :], wx[1][:, :], eqB[:, :], VA.mult)
            nc.vector.tensor_tensor(WL[:, :], WL[:, :], t1[:, :], VA.add)
            nc.vector.tensor_tensor(WR[:, :], wx[1][:, :], eqA[:, :], VA.mult)
            nc.vector.tensor_tensor(t1[:, :], wx[0][:, :], eqC[:, :], VA.mult)
            nc.vector.tensor_tensor(WR[:, :], WR[:, :], t1[:, :], VA.add)
            # weights per (a, side): wcor2[a*2+side]
            wcor2 = []
            for a in range(2):
                for sd, Wside in ((0, WL), (1, WR)):
                    wc, _ = f3(f"wc{a}{sd}")
                    nc.vector.tensor_tensor(wc[:, :], wy[a][:, :], Wside[:, :], VA.mult)
                    wcor2.append(wc)
            # pair row indices idx = cy*64 + bx; fidx col = k*64+g*32+a*16+cl
            cys = []
            for a in range(2):
                cs, _ = f3(f"cys{a}")
                nc.vector.tensor_scalar_mul(cs[:, :], cy[a][:, :], float(W))
                cys.append(cs)
            fidx = fld.tile([128, 2 * FDIM], F32, name="fidx")
            fidx_r = fidx[:, :].rearrange("p (k g a cl) -> p a g cl k",
                                          k=K2, g=NG, a=2, cl=CLG)
            for a in range(2):
                nc.vector.tensor_tensor(fidx_r[:, a],
                                        cys[a][:, :].rearrange(
                                            "p (g cl k) -> p g cl k",
                                            g=NG, cl=CLG, k=K2),
                                        bx[:, :].rearrange(
                                            "p (g cl k) -> p g cl k",
                                            g=NG, cl=CLG, k=K2), VA.add)
            fidxi = fld.tile([128, 2 * FDIM], I16, name="fidxi")
            nc.vector.tensor_copy(fidxi[:, :], fidx[:, :])

            # ---- fold indices into SWDGE wrapped layout ----
            # idxw col = k*1024 + g*512 + m*128 + cl*8 + f ; value stream for
            # (k,g): i = m*2048 + cl*128 + 16f + p'  ->  (i%16, i//16)
            idxw = fld.tile([128, K2 * NG * 2 * CLG * 8], I16, name="idxw")
            src_r = fidxi[:, :].rearrange("p (k gacl) -> p k gacl",
                                          k=K2, gacl=64)
            dst_r = idxw[:, :].rearrange("p (k gacl f) -> p f k gacl",
                                         k=K2, gacl=64, f=8)
            # ACT HWDGE ring: keeps these off the SP FIFO, which is
            # head-of-line blocked by y-write DMAs waiting on ACT copies.
            for f in range(8):
                nc.scalar.dma_start(dst_r[0:16, f],
                                    src_r[16 * f:16 * (f + 1)])
            for f in range(1, 8):
                nc.scalar.dma_start(idxw[16 * f:16 * (f + 1), :], idxw[0:16, :])

            psp_cm = tc.tile_pool(name="ps", bufs=2, space="PSUM")
            psp = psp_cm.__enter__()
            # ---- per-tap 1x1 convs:  yT[pix, (k,o)] = x_chunk^T @ wmain ----
            for c in range(NCH):
                lhs = xin_sb[:, c * 128:(c + 1) * 128]
                y_sb = sb.tile([128, K2 * O], BF16, name=f"ysb{c}", tag="ysb")
                for j in range(3):
                    y_ps = psp.tile([128, 384], F32, name=f"yps{c}_{j}", tag="yps")
                    nc.tensor.matmul(y_ps[:, :], lhs,
                                     wmain_sb[:, j * 384:(j + 1) * 384],
                                     start=True, stop=True)
                    if ycopy_act or c % 2 == 0:
                        nc.scalar.copy(y_sb[:, j * 384:(j + 1) * 384], y_ps[:, :])
                    else:
                        nc.vector.tensor_copy(y_sb[:, j * 384:(j + 1) * 384], y_ps[:, :])
                nc.sync.dma_start(
                    y_dram[:, c * 128:(c + 1) * 128, :].rearrange("k p o -> p k o"),
                    y_sb[:, :].rearrange("p (k o) -> p k o", o=O))

            # ---- gather + weighted accumulate ----
            psop_cm = tc.tile_pool(name="pso", bufs=1, space="PSUM")
            psop = psop_cm.__enter__()
            for g in range(NG):
                psob = []
                for q in range(CLG // 4):
                    p = psop.tile([128, 512], F32, name=f"pso{g}_{q}", tag=f"pso{g}_{q}")
                    psob.append(p)
                psout = [psob[cl // 4][:, (cl % 4) * O:(cl % 4 + 1) * O]
                         for cl in range(CLG)]
                for k in range(K2):
                    ysrc = y_dram[k, :, :]
                    ypairs = bass.AP(tensor=ysrc.tensor, offset=ysrc.offset,
                                     ap=[[O, HW - 1], [1, 2 * O]])
                    for a in range(2):
                        gt = gth.tile([128, CLG, 2 * O], BF16,
                                      name=f"g{g}_{k}_{a}", tag="gath")
                        BA = CLG * 8
                        base = k * (NG * 2 * BA) + g * (2 * BA) + a * BA
                        for s in range(max(1, CLG // 8)):
                            nc.gpsimd.dma_gather(
                                gt[:, s * 8:(s + 1) * 8, :], ypairs,
                                idxw[:, base + s * 64: base + (s + 1) * 64],
                                1024, 1024, 2 * O, elem_step=O)
                        for sd in range(2):
                            for q in range(CLG // 4):
                                tmp = tmppool.tile([128, 512], BF16,
                                                   name=f"t{g}_{k}_{a}_{sd}_{q}",
                                                   tag="tmp")
                                blk = (k * 8 + a * 4 + sd * 2 + (q & 1)) % act_mod if act_mod else 1
                                for j in range(4):
                                    cl = q * 4 + j
                                    c = g * CLG + cl
                                    if blk == 0:
                                        nc.scalar.activation(
                                            tmp[:, j * O:(j + 1) * O],
                                            gt[:, cl, sd * O:(sd + 1) * O],
                                            mybir.ActivationFunctionType.Copy,
                                            scale=wcor2[a * 2 + sd][:, c * K2 + k:
                                                                    c * K2 + k + 1])
                                    else:
                                        nc.vector.tensor_scalar_mul(
                                            tmp[:, j * O:(j + 1) * O],
                                            gt[:, cl, sd * O:(sd + 1) * O],
                                            wcor2[a * 2 + sd][:, c * K2 + k:
                                                              c * K2 + k + 1])
                                nc.tensor.matmul(psob[q][:, :], identb[:, :],
                                                 tmp[:, :],
                                                 start=(k == 0 and a == 0 and sd == 0),
                                                 stop=(k == K2 - 1 and a == 1 and sd == 1))
                for cl in range(CLG):
                    c = g * CLG + cl
                    ot = sb.tile([128, O], F32, name=f"o{g}_{cl}", tag="ot")
                    nc.vector.tensor_tensor(ot[:, :], psout[cl],
                                            bias_sb[:, :], mybir.AluOpType.add)
                    nc.sync.dma_start(out_dram[c * 128:(c + 1) * 128, :], ot[:, :])
            psop_cm.__exit__(None, None, None)

    nc.compile()
    _split_excess_waits(nc)
    return nc


_NC_CACHE = None


def _get_nc():
    global _NC_CACHE
    if _NC_CACHE is None:
        _NC_CACHE = build_nc()
    return _NC_CACHE


def _host_inputs(x, offset_w, offset_b, weight, bias):
    bf = ml_dtypes.bfloat16
    # constant (shared) tensors
    offw = np.ascontiguousarray(
        offset_w.reshape(18, C, K2).transpose(1, 2, 0).reshape(C, K2 * 18)).astype(bf)
    wmain = np.ascontiguousarray(
        weight.reshape(O, C, K2).transpose(1, 2, 0).reshape(C, K2 * O)).astype(bf)
    offb = offset_b.reshape(18, 1).astype(np.float32)
    bias_t = np.tile(bias.reshape(1, O), (128, 1)).astype(np.float32)
    pi = np.arange(128)
    cc = np.arange(NCH)
    kk = np.arange(K2)
    pix = cc[None, :, None] * 128 + pi[:, None, None]          # [128, 32, 1]
    ybase = (pix // W - 1 + (kk // 3)[None, None, :]).reshape(128, FDIM_np).astype(np.float32)
    xbase = (pix % W - 1 + (kk % 3)[None, None, :]).reshape(128, FDIM_np).astype(np.float32)
    identf = np.eye(128, dtype=np.float32)
    identb = np.eye(128, dtype=bf)
    shared = dict(offw=offw, wmain=wmain, offb=offb, bias_t=bias_t,
                  ybase=ybase, xbase=xbase, identf=identf, identb=identb)
    maps = []
    for b in range(B):
        m = dict(shared)
        m["x_img"] = np.ascontiguousarray(x[b].reshape(C, HW)).astype(bf)
        maps.append(m)
    return maps


FDIM_np = NCH * K2


def kernel(x, offset_w, offset_b, weight, bias):
    from concourse.bass_utils import run_bass_kernel_spmd
    nc = _get_nc()
    in_maps = _host_inputs(np.asarray(x, np.float32), np.asarray(offset_w, np.float32),
                           np.asarray(offset_b, np.float32),
                           np.asarray(weight, np.float32), np.asarray(bias, np.float32))
    res = run_bass_kernel_spmd(nc, in_maps, core_ids=list(range(B)))
    out = np.stack([np.asarray(res.results[b]["out"], np.float32).T.reshape(O, H, W)
                    for b in range(B)])
    return out
